# revision 6
# baseline (speedup 1.0000x reference)
"""BitSelfAttention TRN2 kernel v2 (8 NeuronCores, TP over heads + batch over B).

Core c -> batch b=c//4, head group hg=c%4 (heads 4*hg..4*hg+3).
All matmuls bf16 (1 cy/row); dequant threshold math kept f32-exact:
scale sum via gpsimd partition_all_reduce (f32), compare via DVE STT.
Rope half-swap baked into the DRAM store (swapped row ranges); reload adds
qa + swap(qs) on DVE in bf16.  Softmax denominator via DVE-accumulated exp
sums + one [1,512] matmul per (j,h).  o_proj PSUM evac on Pool engine.
Self-contained; includes the walrus one-wait BIR legalizer.
"""
import json
import numpy as np

# ---------------------------------------------------------------- constants
P = 128
T = 2048
D = 2048
NH = 4                     # heads per core
HD = 128                   # head dim
TB = 512                   # t-block
NTB = T // TB              # 4
G = D // P                 # 16 contraction chunks
OSH = 512                  # per-core qkv output-column shard
SCALE = HD ** -0.5
NEG = -1e30
MEGA = 1                   # dequant chunks merged per op

_cached = {}


# ------------------------------------------------------------- BIR legalizer
def _legalize_bir_json(bir_json: bytes) -> bytes:
    """This walrus accepts only ONE sync-wait (and update) per instruction.
    Hoist extras onto same-engine NoOps (engine FIFO keeps semantics)."""
    m = json.loads(bir_json)
    n = [0]

    def nop(engine, waits, updates):
        n[0] += 1
        return {"name": f"I-wfix{n[0]}", "opcode": "NoOp", "engine": engine,
                "ins": [], "outs": [],
                "sync_info": {"on_wait": waits, "on_update": updates}}

    for f in m.get("functions", []):
        for blk in f.get("blocks", []):
            out = []
            for inst in blk.get("instructions", []):
                si = inst.get("sync_info")
                if not si:
                    out.append(inst)
                    continue
                waits = si.get("on_wait") or []
                ups = si.get("on_update") or []
                post = []
                if len(waits) > 1:
                    for w in waits[:-1]:
                        out.append(nop(inst["engine"], [w], []))
                    si["on_wait"] = [waits[-1]]
                if len(ups) > 1:
                    assert inst.get("opcode") not in (
                        "DMACopy", "DMATranspose", "DMAGather",
                        "DMAScatterAdd", "TriggerDma"), inst.get("name")
                    si["on_update"] = [ups[0]]
                    for u in ups[1:]:
                        post.append(nop(inst["engine"], [], [u]))
                out.append(inst)
                out.extend(post)
            blk["instructions"] = out
    return json.dumps(m).encode()


def _install_waitfix():
    import concourse.bass_utils as bu
    if getattr(bu, "_bitattn_waitfix", False):
        return
    bu._bitattn_waitfix = True
    orig = bu.compile_bir_kernel

    def patched(bir_json, tmpdir, neff_name="file.neff"):
        return orig(_legalize_bir_json(bir_json), tmpdir, neff_name)

    bu.compile_bir_kernel = patched
    try:
        import concourse.bass2jax as b2j
        if getattr(b2j, "compile_bir_kernel", None) is orig:
            b2j.compile_bir_kernel = patched
    except ImportError:
        pass


# ---------------------------------------------------------------- bass build
def _build_nc():
    import concourse.bass as bass
    import concourse.mybir as mybir
    import concourse.tile as tile
    import concourse.bass_isa as bass_isa
    from concourse import library_config
    from contextlib import ExitStack

    F32 = mybir.dt.float32
    BF = mybir.dt.bfloat16
    AF = mybir.ActivationFunctionType
    ALU = mybir.AluOpType
    RED = bass_isa.ReduceOp

    nc = bass.Bass(name="bitattn2", trn_type="TRN2")
    xT_in = nc.dram_tensor("xT", [D, T], BF, kind="ExternalInput")
    wqT_in = nc.dram_tensor("wqT", [D, OSH], F32, kind="ExternalInput")
    wkT_in = nc.dram_tensor("wkT", [D, OSH], F32, kind="ExternalInput")
    wvT_in = nc.dram_tensor("wvT", [D, OSH], F32, kind="ExternalInput")
    woT_in = nc.dram_tensor("woT", [OSH, D], F32, kind="ExternalInput")
    ropeC_in = nc.dram_tensor("ropeC", [P, T], BF, kind="ExternalInput")
    ropeS_in = nc.dram_tensor("ropeS", [P, T], BF, kind="ExternalInput")
    tri_in = nc.dram_tensor("tri", [P, P], BF, kind="ExternalInput")
    outT = nc.dram_tensor("outT", [D, T], BF, kind="ExternalOutput")

    xT_v = xT_in[:].rearrange("(g p) t -> g p t", p=P)
    w_views = {
        "q": wqT_in[:].rearrange("(g p) o -> g p o", p=P),
        "k": wkT_in[:].rearrange("(g p) o -> g p o", p=P),
        "v": wvT_in[:].rearrange("(g p) o -> g p o", p=P),
    }
    woT_v = woT_in[:].rearrange("(fc p) o -> fc p o", p=P)
    outT_v = outT[:].rearrange("(ob p) t -> ob p t", p=P)

    with tile.TileContext(nc) as tc, ExitStack() as ctx:
        nc.gpsimd.load_library(library_config.attn)

        glob = ctx.enter_context(tc.tile_pool(name="glob", bufs=1))
        ones_f = glob.tile([P, 1], F32)
        nc.gpsimd.memset(ones_f[:], 1.0)
        ones_b = glob.tile([P, 1], BF)
        nc.vector.tensor_copy(ones_b[:], ones_f[:])
        tri = glob.tile([P, P], BF)
        ropeC = glob.tile([P, T], BF)
        ropeS = glob.tile([P, T], BF)

        # ---------------- dequant mega-chunk helper --------------------
        # For MEGA groups side by side: [128, MEGA*OSH].  Per column o of
        # group g: s = sum_p(2|w|)/256; keep if 2|w| > s; W_dq = sign(w)*s.
        # partition_all_reduce keeps per-column sums independent, so two
        # groups can share one op along the free dim.
        dq_pend = []

        def dequant_front(dq, src_aps, dst_ap, fw):
            wt = dq.tile([P, fw], F32, tag="wt", name="wt")
            step = fw // len(src_aps)
            for i, ap in enumerate(src_aps):
                nc.sync.dma_start(wt[:, i * step:(i + 1) * step], ap)
            ab2 = dq.tile([P, fw], F32, tag="ab2", name="ab2")
            nc.scalar.activation(ab2[:], wt[:], AF.Abs, scale=2.0)
            sbr = dq.tile([P, fw], F32, tag="sbr", name="sbr")
            nc.gpsimd.partition_all_reduce(sbr[:], ab2[:], channels=P,
                                           reduce_op=RED.add)
            m01 = dq.tile([P, fw], F32, tag="m01", name="m01")
            nc.vector.scalar_tensor_tensor(m01[:], sbr[:], 1.0 / 256.0,
                                           ab2[:], ALU.mult, ALU.is_lt)
            sgn = dq.tile([P, fw], BF, tag="sgn", name="sgn")
            nc.scalar.activation(sgn[:], wt[:], AF.Sign)
            dq_pend.append((dq, sbr, m01, sgn, dst_ap, fw))

        def dequant_back():
            dq, sbr, m01, sgn, dst_ap, fw = dq_pend.pop(0)
            ms = dq.tile([P, fw], BF, tag="ms", name="ms")
            nc.vector.scalar_tensor_tensor(ms[:], sbr[:], 1.0 / 256.0,
                                           m01[:], ALU.mult, ALU.mult)
            nc.vector.tensor_tensor(dst_ap, ms[:], sgn[:], ALU.mult)

        # emit back-half one chunk late: DVE never queues a mul that waits
        # on the same chunk's Pool ms (head-of-line stall)
        def dequant_mega(dq, src_aps, dst_ap, fw):
            dequant_front(dq, src_aps, dst_ap, fw)
            if len(dq_pend) > 1:
                dequant_back()

        # attention operands built in SBUF during phase A — no DRAM round trip
        apool = ctx.enter_context(tc.tile_pool(name="apool", bufs=1))
        khs = apool.tile([P, NH, T], BF, name="khs")
        vhs = apool.tile([P, G, NH * HD], BF, name="vhs")   # [keys, kb, 4h*d]
        y_sb = apool.tile([P, NH, T], BF, name="y_sb")
        qj_all = apool.tile([P, NTB, NH, TB], BF, name="qj_all")

        wopool = ctx.enter_context(tc.tile_pool(name="wop", bufs=1))
        wo_dq = wopool.tile([P, NH, D], BF, name="wo_dq")

        # ================= phase A: dequant + QKV + rope ===============
        with ExitStack() as pctx:
            wpool = pctx.enter_context(tc.tile_pool(name="wdqp", bufs=1))
            w_dq = {
                "q": wpool.tile([P, G, OSH], BF, name="wq_dq"),
                "k": wpool.tile([P, G, OSH], BF, name="wk_dq"),
                "v": wpool.tile([P, G, OSH], BF, name="wv_dq"),
            }
            dq = pctx.enter_context(tc.tile_pool(name="dq", bufs=3))
            first = [True]

            def dequant_all(pr):
                for m in range(G // MEGA):
                    gs = [m * MEGA + i for i in range(MEGA)]
                    dequant_mega(
                        dq, [w_views[pr][g] for g in gs],
                        w_dq[pr][:, gs[0]:gs[-1] + 1].rearrange(
                            "p g o -> p (g o)"),
                        MEGA * OSH)
                    if first[0]:
                        # tables behind the first w chunk in the DMA queue
                        first[0] = False
                        nc.sync.dma_start(tri[:], tri_in[:])
                        nc.sync.dma_start(ropeC[:], ropeC_in[:])
                        nc.sync.dma_start(ropeS[:], ropeS_in[:])

            for pr in ("q", "k", "v"):
                dequant_all(pr)
            # wo last in the stream: ready long before o_proj consumes it
            for fc in range(NH):
                for m in range(4 // MEGA):
                    oc0 = m * MEGA * OSH
                    dequant_mega(
                        dq,
                        [woT_v[fc, :, oc0 + i * OSH:oc0 + (i + 1) * OSH]
                         for i in range(MEGA)],
                        wo_dq[:, fc, oc0:oc0 + MEGA * OSH], MEGA * OSH)
            while dq_pend:
                dequant_back()

            xpool = pctx.enter_context(tc.tile_pool(name="xp", bufs=2))
            evac = pctx.enter_context(tc.tile_pool(name="evac", bufs=2))
            psQK = pctx.enter_context(
                tc.tile_pool(name="psQK", bufs=1, space="PSUM"))
            psV = pctx.enter_context(
                tc.tile_pool(name="psV", bufs=2, space="PSUM"))

            for tb in range(NTB):
                ts = slice(tb * TB, (tb + 1) * TB)
                xTr = xpool.tile([P, G, TB], BF, tag="xTr", name="xTr")
                for g in range(G):
                    nc.sync.dma_start(xTr[:, g], xT_v[g, :, ts])

                # q, k projections: g-outer / head-inner so PE consumption
                # paces the dequant stream on tb=0.
                for pr in ("q", "k"):
                    pqs = [psQK.tile([P, TB], F32, tag=f"pq{h}", name=f"pq{h}")
                           for h in range(NH)]
                    for g in range(G):
                        for h in range(NH):
                            nc.tensor.matmul(
                                pqs[h][:], w_dq[pr][:, g, h * HD:(h + 1) * HD],
                                xTr[:, g], start=(g == 0), stop=(g == G - 1))
                    for h in range(NH):
                        qe = evac.tile([P, TB], BF, tag="qe", name="qe")
                        nc.scalar.copy(qe[:], pqs[h][:])
                        qa = evac.tile([P, TB], BF, tag="qa", name="qa")
                        nc.vector.tensor_tensor(qa[:], qe[:],
                                                ropeC[:, ts], ALU.mult)
                        qs = evac.tile([P, TB], BF, tag="qs", name="qs")
                        nc.vector.tensor_tensor(qs[:], qe[:],
                                                ropeS[:, ts], ALU.mult)
                        # rope half-swap: SBUF->SBUF DMA (engines can't cross
                        # partition offsets), then one same-partition add
                        tsw = evac.tile([P, TB], BF, tag="tsw", name="tsw")
                        nc.sync.dma_start(tsw[0:64], qs[64:128])
                        nc.sync.dma_start(tsw[64:128], qs[0:64])
                        if pr == "q":
                            dst = qj_all[:, tb, h]
                        else:
                            dst = khs[:, h, tb * TB:(tb + 1) * TB]
                        nc.vector.tensor_tensor(dst, qa[:], tsw[:], ALU.add)

                # v projection, output directly [keys, kb, vdims] in SBUF
                for tk in range(NTB):
                    pv = psV.tile([P, TB], F32, tag="pv", name="pv")
                    for g in range(G):
                        nc.tensor.matmul(
                            pv[:], xTr[:, g, tk * HD:(tk + 1) * HD],
                            w_dq["v"][:, g], start=(g == 0), stop=(g == G - 1))
                    nc.scalar.copy(vhs[:, tb * NTB + tk], pv[:])

        # ============ phase B: attention + o_proj ======================
        with ExitStack() as pctx:
            expool = pctx.enter_context(tc.tile_pool(name="exp", bufs=4))
            sspool = pctx.enter_context(tc.tile_pool(name="ss", bufs=3))
            opool = pctx.enter_context(tc.tile_pool(name="op", bufs=4))
            psS = pctx.enter_context(
                tc.tile_pool(name="psS", bufs=3, space="PSUM"))
            psY = pctx.enter_context(
                tc.tile_pool(name="psY", bufs=2, space="PSUM"))
            psD = pctx.enter_context(
                tc.tile_pool(name="psD", bufs=1, space="PSUM"))
            psO = pctx.enter_context(
                tc.tile_pool(name="psO", bufs=2, space="PSUM"))

            def oproj_tb(tb):
                ts = slice(tb * TB, (tb + 1) * TB)
                for ob in range(G):
                    ps_o = psO.tile([P, TB], F32, tag="ps_o", name="ps_o")
                    for fc in range(NH):
                        nc.tensor.matmul(
                            ps_o[:], wo_dq[:, fc, ob * P:(ob + 1) * P],
                            y_sb[:, fc, ts],
                            start=(fc == 0), stop=(fc == NH - 1))
                    ot = opool.tile([P, TB], BF, tag="ot", name="ot")
                    if ob % 2 == 0:
                        nc.scalar.copy(ot[:], ps_o[:])
                    else:
                        nc.vector.tensor_copy(ot[:], ps_o[:])
                    nc.sync.dma_start(outT_v[ob, :, ts], ot[:])

            for j in range(NTB):
                ts = slice(j * TB, (j + 1) * TB)
                nkk = 4 * j + 4
                for h in range(NH):
                    qj = qj_all[:, j, h]
                    ps_y = psY.tile([P, TB], F32, tag="py", name="py")
                    exs = sspool.tile([P, TB], BF, tag="exs", name="exs")
                    pend = []  # (kk, ex, off, ncols)

                    def flush_pv(kk, ex, off, ncols):
                        nc.tensor.matmul(ps_y[:, off:],
                                         vhs[:, kk, h * HD:(h + 1) * HD],
                                         ex[:, 0:ncols],
                                         start=(kk == 0), stop=(kk == nkk - 1))

                    for kk in range(nkk):
                        d = kk - 4 * j
                        off = P * d if d >= 0 else 0
                        ncols = TB - off
                        ps_st = psS.tile([P, TB], F32, tag="st", name="st")
                        st = ps_st[:, 0:ncols]
                        nc.tensor.matmul(
                            st, khs[:, h, kk * P:(kk + 1) * P],
                            qj[:, off:TB], start=True, stop=True)
                        if d >= 0:
                            nc.vector.tensor_tensor(ps_st[:, 0:P],
                                                    ps_st[:, 0:P], tri[:],
                                                    ALU.add)
                        ex = expool.tile([P, TB], BF, tag="ex", name="ex")
                        nc.scalar.activation(ex[:, 0:ncols], st, AF.Exp,
                                             scale=SCALE)
                        if kk == 0:
                            nc.vector.tensor_copy(exs[:], ex[:])
                        else:
                            nc.vector.tensor_tensor(
                                exs[:, off:], exs[:, off:], ex[:, 0:ncols],
                                ALU.add)
                        pend.append((kk, ex, off, ncols))
                        if len(pend) > 2:
                            flush_pv(*pend.pop(0))
                    while pend:
                        flush_pv(*pend.pop(0))

                    ps_den = psD.tile([1, TB], F32, tag="pd", name="pd")
                    nc.tensor.matmul(ps_den[:], ones_b[:], exs[:],
                                     start=True, stop=True)
                    rec = expool.tile([1, TB], BF, tag="rec", name="rec")
                    with nc.allow_low_precision("bf16 1/denom"):
                        nc.vector.reciprocal(rec[:], ps_den[:])
                    den_b = sspool.tile([P, TB], BF, tag="den_b",
                                        name="den_b")
                    nc.gpsimd.partition_broadcast(den_b[:], rec[:])
                    nc.vector.tensor_tensor(y_sb[:, h, ts], ps_y[:],
                                            den_b[:], ALU.mult)
                oproj_tb(j)

    from concourse import library_overlay
    library_overlay.lower_extended_insts(nc)
    return nc


def _rope_tables():
    half = HD // 2
    inv_freq = 1.0 / (10000.0 ** (np.arange(half, dtype=np.float64) / half))
    freqs = np.outer(np.arange(T, dtype=np.float64), inv_freq)  # [T, 64]
    c = np.cos(freqs).T  # [64, T]
    s = np.sin(freqs).T
    # S'' = [sin; -sin]: rope = q*C + swap_halves(q*S'')
    return (np.concatenate([c, c], axis=0),
            np.concatenate([s, -s], axis=0))


def kernel(x, w_q, w_k, w_v, w_o):
    _install_waitfix()
    from concourse.bass_utils import run_bass_kernel_spmd
    import ml_dtypes

    bf16 = ml_dtypes.bfloat16
    x = np.asarray(x, dtype=np.float32)
    w_q = np.asarray(w_q, dtype=np.float32)
    w_k = np.asarray(w_k, dtype=np.float32)
    w_v = np.asarray(w_v, dtype=np.float32)
    w_o = np.asarray(w_o, dtype=np.float32)
    B = x.shape[0]

    if "nc" not in _cached:
        _cached["nc"] = _build_nc()
    nc = _cached["nc"]

    ropeC, ropeS = _rope_tables()
    idx = np.arange(P)
    tri = np.where(idx[:, None] > idx[None, :], np.float32(NEG),
                   np.float32(0.0))

    in_maps = []
    for c in range(8):
        b, hg = divmod(c, 4)
        osl = slice(hg * OSH, (hg + 1) * OSH)
        in_maps.append({
            "xT": np.ascontiguousarray(x[b].T).astype(bf16),
            "wqT": np.ascontiguousarray(w_q[osl, :].T),
            "wkT": np.ascontiguousarray(w_k[osl, :].T),
            "wvT": np.ascontiguousarray(w_v[osl, :].T),
            "woT": np.ascontiguousarray(w_o[:, osl].T),
            "ropeC": ropeC.astype(bf16), "ropeS": ropeS.astype(bf16),
            "tri": tri.astype(bf16),
        })

    import os as _os
    trace = _os.environ.get("BITATTN_TRACE") == "1"
    res = run_bass_kernel_spmd(nc, in_maps, core_ids=list(range(8)),
                               trace=trace)
    _cached["last_res"] = res
    out = np.zeros((B, T, D), dtype=np.float32)
    for c in range(8):
        b = c // 4
        out[b] += res.results[c]["outT"].astype(np.float32).T
    return out


# revision 7
# speedup vs baseline: 1.0039x; 1.0039x over previous
"""BitSelfAttention TRN2 kernel v2 (8 NeuronCores, TP over heads + batch over B).

Core c -> batch b=c//4, head group hg=c%4 (heads 4*hg..4*hg+3).
All matmuls bf16 (1 cy/row); dequant threshold math kept f32-exact:
scale sum via gpsimd partition_all_reduce (f32), compare via DVE STT.
Rope half-swap baked into the DRAM store (swapped row ranges); reload adds
qa + swap(qs) on DVE in bf16.  Softmax denominator via DVE-accumulated exp
sums + one [1,512] matmul per (j,h).  o_proj PSUM evac on Pool engine.
Self-contained; includes the walrus one-wait BIR legalizer.
"""
import json
import numpy as np

# ---------------------------------------------------------------- constants
P = 128
T = 2048
D = 2048
NH = 4                     # heads per core
HD = 128                   # head dim
TB = 512                   # t-block
NTB = T // TB              # 4
G = D // P                 # 16 contraction chunks
OSH = 512                  # per-core qkv output-column shard
SCALE = HD ** -0.5
NEG = -1e30
MEGA = 1                   # dequant chunks merged per op

_cached = {}


# ------------------------------------------------------------- BIR legalizer
def _legalize_bir_json(bir_json: bytes) -> bytes:
    """This walrus accepts only ONE sync-wait (and update) per instruction.
    Hoist extras onto same-engine NoOps (engine FIFO keeps semantics)."""
    m = json.loads(bir_json)
    n = [0]

    def nop(engine, waits, updates):
        n[0] += 1
        return {"name": f"I-wfix{n[0]}", "opcode": "NoOp", "engine": engine,
                "ins": [], "outs": [],
                "sync_info": {"on_wait": waits, "on_update": updates}}

    for f in m.get("functions", []):
        for blk in f.get("blocks", []):
            out = []
            for inst in blk.get("instructions", []):
                si = inst.get("sync_info")
                if not si:
                    out.append(inst)
                    continue
                waits = si.get("on_wait") or []
                ups = si.get("on_update") or []
                post = []
                if len(waits) > 1:
                    for w in waits[:-1]:
                        out.append(nop(inst["engine"], [w], []))
                    si["on_wait"] = [waits[-1]]
                if len(ups) > 1:
                    assert inst.get("opcode") not in (
                        "DMACopy", "DMATranspose", "DMAGather",
                        "DMAScatterAdd", "TriggerDma"), inst.get("name")
                    si["on_update"] = [ups[0]]
                    for u in ups[1:]:
                        post.append(nop(inst["engine"], [], [u]))
                out.append(inst)
                out.extend(post)
            blk["instructions"] = out
    return json.dumps(m).encode()


def _install_waitfix():
    import concourse.bass_utils as bu
    if getattr(bu, "_bitattn_waitfix", False):
        return
    bu._bitattn_waitfix = True
    orig = bu.compile_bir_kernel

    def patched(bir_json, tmpdir, neff_name="file.neff"):
        return orig(_legalize_bir_json(bir_json), tmpdir, neff_name)

    bu.compile_bir_kernel = patched
    try:
        import concourse.bass2jax as b2j
        if getattr(b2j, "compile_bir_kernel", None) is orig:
            b2j.compile_bir_kernel = patched
    except ImportError:
        pass


# ---------------------------------------------------------------- bass build
def _build_nc():
    import concourse.bass as bass
    import concourse.mybir as mybir
    import concourse.tile as tile
    import concourse.bass_isa as bass_isa
    from concourse import library_config
    from contextlib import ExitStack

    F32 = mybir.dt.float32
    BF = mybir.dt.bfloat16
    AF = mybir.ActivationFunctionType
    ALU = mybir.AluOpType
    RED = bass_isa.ReduceOp

    nc = bass.Bass(name="bitattn2", trn_type="TRN2")
    xT_in = nc.dram_tensor("xT", [D, T], BF, kind="ExternalInput")
    wqT_in = nc.dram_tensor("wqT", [D, OSH], F32, kind="ExternalInput")
    wkT_in = nc.dram_tensor("wkT", [D, OSH], F32, kind="ExternalInput")
    wvT_in = nc.dram_tensor("wvT", [D, OSH], F32, kind="ExternalInput")
    woT_in = nc.dram_tensor("woT", [OSH, D], F32, kind="ExternalInput")
    ropeC_in = nc.dram_tensor("ropeC", [P, T], BF, kind="ExternalInput")
    ropeS_in = nc.dram_tensor("ropeS", [P, T], BF, kind="ExternalInput")
    tri_in = nc.dram_tensor("tri", [P, P], BF, kind="ExternalInput")
    outT = nc.dram_tensor("outT", [D, T], BF, kind="ExternalOutput")

    xT_v = xT_in[:].rearrange("(g p) t -> g p t", p=P)
    w_views = {
        "q": wqT_in[:].rearrange("(g p) o -> g p o", p=P),
        "k": wkT_in[:].rearrange("(g p) o -> g p o", p=P),
        "v": wvT_in[:].rearrange("(g p) o -> g p o", p=P),
    }
    woT_v = woT_in[:].rearrange("(fc p) o -> fc p o", p=P)
    outT_v = outT[:].rearrange("(ob p) t -> ob p t", p=P)

    with tile.TileContext(nc) as tc, ExitStack() as ctx:
        nc.gpsimd.load_library(library_config.attn)

        glob = ctx.enter_context(tc.tile_pool(name="glob", bufs=1))
        ones_f = glob.tile([P, 1], F32)
        nc.gpsimd.memset(ones_f[:], 1.0)
        ones_b = glob.tile([P, 1], BF)
        nc.vector.tensor_copy(ones_b[:], ones_f[:])
        tri = glob.tile([P, P], BF)
        ropeC = glob.tile([P, T], BF)
        ropeS = glob.tile([P, T], BF)

        # ---------------- dequant mega-chunk helper --------------------
        # For MEGA groups side by side: [128, MEGA*OSH].  Per column o of
        # group g: s = sum_p(2|w|)/256; keep if 2|w| > s; W_dq = sign(w)*s.
        # partition_all_reduce keeps per-column sums independent, so two
        # groups can share one op along the free dim.
        dq_pend = []

        def dequant_front(dq, src_aps, dst_ap, fw):
            wt = dq.tile([P, fw], F32, tag="wt", name="wt")
            step = fw // len(src_aps)
            for i, ap in enumerate(src_aps):
                nc.sync.dma_start(wt[:, i * step:(i + 1) * step], ap)
            ab2 = dq.tile([P, fw], F32, tag="ab2", name="ab2")
            nc.scalar.activation(ab2[:], wt[:], AF.Abs, scale=2.0)
            sbr = dq.tile([P, fw], F32, tag="sbr", name="sbr")
            nc.gpsimd.partition_all_reduce(sbr[:], ab2[:], channels=P,
                                           reduce_op=RED.add)
            m01 = dq.tile([P, fw], F32, tag="m01", name="m01")
            nc.vector.scalar_tensor_tensor(m01[:], sbr[:], 1.0 / 256.0,
                                           ab2[:], ALU.mult, ALU.is_lt)
            sgn = dq.tile([P, fw], BF, tag="sgn", name="sgn")
            nc.scalar.activation(sgn[:], wt[:], AF.Sign)
            dq_pend.append((dq, sbr, m01, sgn, dst_ap, fw))

        dq_n = [0]

        def dequant_back():
            dq, sbr, m01, sgn, dst_ap, fw = dq_pend.pop(0)
            ms = dq.tile([P, fw], BF, tag="ms", name="ms")
            nc.vector.scalar_tensor_tensor(ms[:], sbr[:], 1.0 / 256.0,
                                           m01[:], ALU.mult, ALU.mult)
            nc.vector.tensor_tensor(dst_ap, ms[:], sgn[:], ALU.mult)

        # emit back-half one chunk late: DVE never queues a mul that waits
        # on the same chunk's Pool ms (head-of-line stall)
        def dequant_mega(dq, src_aps, dst_ap, fw):
            dequant_front(dq, src_aps, dst_ap, fw)
            if len(dq_pend) > 1:
                dequant_back()

        # attention operands built in SBUF during phase A — no DRAM round trip
        apool = ctx.enter_context(tc.tile_pool(name="apool", bufs=1))
        khs = apool.tile([P, NH, T], BF, name="khs")
        vhs = apool.tile([P, G, NH * HD], BF, name="vhs")   # [keys, kb, 4h*d]
        y_sb = apool.tile([P, NH, T], BF, name="y_sb")
        qj_all = apool.tile([P, NTB, NH, TB], BF, name="qj_all")

        wopool = ctx.enter_context(tc.tile_pool(name="wop", bufs=1))
        wo_dq = wopool.tile([P, NH, D], BF, name="wo_dq")

        # ================= phase A: dequant + QKV + rope ===============
        with ExitStack() as pctx:
            wpool = pctx.enter_context(tc.tile_pool(name="wdqp", bufs=1))
            w_dq = {
                "q": wpool.tile([P, G, OSH], BF, name="wq_dq"),
                "k": wpool.tile([P, G, OSH], BF, name="wk_dq"),
                "v": wpool.tile([P, G, OSH], BF, name="wv_dq"),
            }
            dq = pctx.enter_context(tc.tile_pool(name="dq", bufs=3))
            first = [True]

            def dequant_all(pr):
                for m in range(G // MEGA):
                    gs = [m * MEGA + i for i in range(MEGA)]
                    dequant_mega(
                        dq, [w_views[pr][g] for g in gs],
                        w_dq[pr][:, gs[0]:gs[-1] + 1].rearrange(
                            "p g o -> p (g o)"),
                        MEGA * OSH)
                    if first[0]:
                        # tables behind the first w chunk in the DMA queue
                        first[0] = False
                        nc.sync.dma_start(tri[:], tri_in[:])
                        nc.sync.dma_start(ropeC[:], ropeC_in[:])
                        nc.sync.dma_start(ropeS[:], ropeS_in[:])

            for pr in ("q", "k", "v"):
                dequant_all(pr)
            # wo last in the stream: ready long before o_proj consumes it
            for fc in range(NH):
                for m in range(4 // MEGA):
                    oc0 = m * MEGA * OSH
                    dequant_mega(
                        dq,
                        [woT_v[fc, :, oc0 + i * OSH:oc0 + (i + 1) * OSH]
                         for i in range(MEGA)],
                        wo_dq[:, fc, oc0:oc0 + MEGA * OSH], MEGA * OSH)
            while dq_pend:
                dequant_back()

            xpool = pctx.enter_context(tc.tile_pool(name="xp", bufs=2))
            evac = pctx.enter_context(tc.tile_pool(name="evac", bufs=2))
            psQK = pctx.enter_context(
                tc.tile_pool(name="psQK", bufs=1, space="PSUM"))
            psV = pctx.enter_context(
                tc.tile_pool(name="psV", bufs=2, space="PSUM"))

            for tb in range(NTB):
                ts = slice(tb * TB, (tb + 1) * TB)
                xTr = xpool.tile([P, G, TB], BF, tag="xTr", name="xTr")
                for g in range(G):
                    nc.sync.dma_start(xTr[:, g], xT_v[g, :, ts])

                # q, k projections: g-outer / head-inner so PE consumption
                # paces the dequant stream on tb=0.
                for pr in ("q", "k"):
                    pqs = [psQK.tile([P, TB], F32, tag=f"pq{h}", name=f"pq{h}")
                           for h in range(NH)]
                    for g in range(G):
                        for h in range(NH):
                            nc.tensor.matmul(
                                pqs[h][:], w_dq[pr][:, g, h * HD:(h + 1) * HD],
                                xTr[:, g], start=(g == 0), stop=(g == G - 1))
                    for h in range(NH):
                        qe = evac.tile([P, TB], BF, tag="qe", name="qe")
                        nc.scalar.copy(qe[:], pqs[h][:])
                        qa = evac.tile([P, TB], BF, tag="qa", name="qa")
                        nc.vector.tensor_tensor(qa[:], qe[:],
                                                ropeC[:, ts], ALU.mult)
                        qs = evac.tile([P, TB], BF, tag="qs", name="qs")
                        nc.vector.tensor_tensor(qs[:], qe[:],
                                                ropeS[:, ts], ALU.mult)
                        # rope half-swap: SBUF->SBUF DMA (engines can't cross
                        # partition offsets), then one same-partition add
                        tsw = evac.tile([P, TB], BF, tag="tsw", name="tsw")
                        nc.sync.dma_start(tsw[0:64], qs[64:128])
                        nc.sync.dma_start(tsw[64:128], qs[0:64])
                        if pr == "q":
                            dst = qj_all[:, tb, h]
                        else:
                            dst = khs[:, h, tb * TB:(tb + 1) * TB]
                        nc.vector.tensor_tensor(dst, qa[:], tsw[:], ALU.add)

                # v projection, output directly [keys, kb, vdims] in SBUF
                for tk in range(NTB):
                    pv = psV.tile([P, TB], F32, tag="pv", name="pv")
                    for g in range(G):
                        nc.tensor.matmul(
                            pv[:], xTr[:, g, tk * HD:(tk + 1) * HD],
                            w_dq["v"][:, g], start=(g == 0), stop=(g == G - 1))
                    nc.scalar.copy(vhs[:, tb * NTB + tk], pv[:])

        # ============ phase B: attention + o_proj ======================
        with ExitStack() as pctx:
            expool = pctx.enter_context(tc.tile_pool(name="exp", bufs=4))
            sspool = pctx.enter_context(tc.tile_pool(name="ss", bufs=3))
            opool = pctx.enter_context(tc.tile_pool(name="op", bufs=4))
            psS = pctx.enter_context(
                tc.tile_pool(name="psS", bufs=3, space="PSUM"))
            psY = pctx.enter_context(
                tc.tile_pool(name="psY", bufs=2, space="PSUM"))
            psD = pctx.enter_context(
                tc.tile_pool(name="psD", bufs=1, space="PSUM"))
            psO = pctx.enter_context(
                tc.tile_pool(name="psO", bufs=2, space="PSUM"))

            def oproj_tb(tb):
                ts = slice(tb * TB, (tb + 1) * TB)
                for ob in range(G):
                    ps_o = psO.tile([P, TB], F32, tag="ps_o", name="ps_o")
                    for fc in range(NH):
                        nc.tensor.matmul(
                            ps_o[:], wo_dq[:, fc, ob * P:(ob + 1) * P],
                            y_sb[:, fc, ts],
                            start=(fc == 0), stop=(fc == NH - 1))
                    ot = opool.tile([P, TB], BF, tag="ot", name="ot")
                    if ob % 2 == 0:
                        nc.scalar.copy(ot[:], ps_o[:])
                    else:
                        nc.vector.tensor_copy(ot[:], ps_o[:])
                    nc.sync.dma_start(outT_v[ob, :, ts], ot[:])

            for j in range(NTB):
                ts = slice(j * TB, (j + 1) * TB)
                nkk = 4 * j + 4
                for h in range(NH):
                    qj = qj_all[:, j, h]
                    ps_y = psY.tile([P, TB], F32, tag="py", name="py")
                    exs = sspool.tile([P, TB], BF, tag="exs", name="exs")
                    pend = []  # (kk, ex, off, ncols)

                    def flush_pv(kk, ex, off, ncols):
                        nc.tensor.matmul(ps_y[:, off:],
                                         vhs[:, kk, h * HD:(h + 1) * HD],
                                         ex[:, 0:ncols],
                                         start=(kk == 0), stop=(kk == nkk - 1))

                    for kk in range(nkk):
                        d = kk - 4 * j
                        off = P * d if d >= 0 else 0
                        ncols = TB - off
                        ps_st = psS.tile([P, TB], F32, tag="st", name="st")
                        st = ps_st[:, 0:ncols]
                        nc.tensor.matmul(
                            st, khs[:, h, kk * P:(kk + 1) * P],
                            qj[:, off:TB], start=True, stop=True)
                        if d >= 0:
                            nc.vector.tensor_tensor(ps_st[:, 0:P],
                                                    ps_st[:, 0:P], tri[:],
                                                    ALU.add)
                        ex = expool.tile([P, TB], BF, tag="ex", name="ex")
                        nc.scalar.activation(ex[:, 0:ncols], st, AF.Exp,
                                             scale=SCALE)
                        if kk == 0:
                            nc.vector.tensor_copy(exs[:], ex[:])
                        else:
                            nc.vector.tensor_tensor(
                                exs[:, off:], exs[:, off:], ex[:, 0:ncols],
                                ALU.add)
                        pend.append((kk, ex, off, ncols))
                        if len(pend) > 2:
                            flush_pv(*pend.pop(0))
                    while pend:
                        flush_pv(*pend.pop(0))

                    ps_den = psD.tile([1, TB], F32, tag="pd", name="pd")
                    nc.tensor.matmul(ps_den[:], ones_b[:], exs[:],
                                     start=True, stop=True)
                    rec = expool.tile([1, TB], BF, tag="rec", name="rec")
                    with nc.allow_low_precision("bf16 1/denom"):
                        nc.vector.reciprocal(rec[:], ps_den[:])
                    den_b = sspool.tile([P, TB], BF, tag="den_b",
                                        name="den_b")
                    nc.gpsimd.partition_broadcast(den_b[:], rec[:])
                    nc.vector.tensor_tensor(y_sb[:, h, ts], ps_y[:],
                                            den_b[:], ALU.mult)
                oproj_tb(j)

    from concourse import library_overlay
    library_overlay.lower_extended_insts(nc)
    return nc


def _rope_tables():
    half = HD // 2
    inv_freq = 1.0 / (10000.0 ** (np.arange(half, dtype=np.float64) / half))
    freqs = np.outer(np.arange(T, dtype=np.float64), inv_freq)  # [T, 64]
    c = np.cos(freqs).T  # [64, T]
    s = np.sin(freqs).T
    # S'' = [sin; -sin]: rope = q*C + swap_halves(q*S'')
    return (np.concatenate([c, c], axis=0),
            np.concatenate([s, -s], axis=0))


def kernel(x, w_q, w_k, w_v, w_o):
    _install_waitfix()
    from concourse.bass_utils import run_bass_kernel_spmd
    import ml_dtypes

    bf16 = ml_dtypes.bfloat16
    x = np.asarray(x, dtype=np.float32)
    w_q = np.asarray(w_q, dtype=np.float32)
    w_k = np.asarray(w_k, dtype=np.float32)
    w_v = np.asarray(w_v, dtype=np.float32)
    w_o = np.asarray(w_o, dtype=np.float32)
    B = x.shape[0]

    if "nc" not in _cached:
        _cached["nc"] = _build_nc()
    nc = _cached["nc"]

    ropeC, ropeS = _rope_tables()
    idx = np.arange(P)
    tri = np.where(idx[:, None] > idx[None, :], np.float32(NEG),
                   np.float32(0.0))

    in_maps = []
    for c in range(8):
        b, hg = divmod(c, 4)
        osl = slice(hg * OSH, (hg + 1) * OSH)
        in_maps.append({
            "xT": np.ascontiguousarray(x[b].T).astype(bf16),
            "wqT": np.ascontiguousarray(w_q[osl, :].T),
            "wkT": np.ascontiguousarray(w_k[osl, :].T),
            "wvT": np.ascontiguousarray(w_v[osl, :].T),
            "woT": np.ascontiguousarray(w_o[:, osl].T),
            "ropeC": ropeC.astype(bf16), "ropeS": ropeS.astype(bf16),
            "tri": tri.astype(bf16),
        })

    import os as _os
    trace = _os.environ.get("BITATTN_TRACE") == "1"
    res = run_bass_kernel_spmd(nc, in_maps, core_ids=list(range(8)),
                               trace=trace)
    _cached["last_res"] = res
    out = np.zeros((B, T, D), dtype=np.float32)
    for c in range(8):
        b = c // 4
        out[b] += res.results[c]["outT"].astype(np.float32).T
    return out


# revision 8
# speedup vs baseline: 62.9087x; 62.6653x over previous
"""BitSelfAttention TRN2 kernel v2 (8 NeuronCores, TP over heads + batch over B).

Core c -> batch b=c//4, head group hg=c%4 (heads 4*hg..4*hg+3).
All matmuls bf16 (1 cy/row); dequant threshold math kept f32-exact:
scale sum via gpsimd partition_all_reduce (f32), compare via DVE STT.
Rope half-swap baked into the DRAM store (swapped row ranges); reload adds
qa + swap(qs) on DVE in bf16.  Softmax denominator via DVE-accumulated exp
sums + one [1,512] matmul per (j,h).  o_proj PSUM evac on Pool engine.
Self-contained; includes the walrus one-wait BIR legalizer.
"""
import json
import numpy as np

# ---------------------------------------------------------------- constants
P = 128
T = 2048
D = 2048
NH = 4                     # heads per core
HD = 128                   # head dim
TB = 512                   # t-block
NTB = T // TB              # 4
G = D // P                 # 16 contraction chunks
OSH = 512                  # per-core qkv output-column shard
SCALE = HD ** -0.5
NEG = -1e30
MEGA = 1                   # dequant chunks merged per op

_cached = {}


# ------------------------------------------------------------- BIR legalizer
def _legalize_bir_json(bir_json: bytes) -> bytes:
    """This walrus accepts only ONE sync-wait (and update) per instruction.
    Hoist extras onto same-engine NoOps (engine FIFO keeps semantics)."""
    m = json.loads(bir_json)
    n = [0]

    def nop(engine, waits, updates):
        n[0] += 1
        return {"name": f"I-wfix{n[0]}", "opcode": "NoOp", "engine": engine,
                "ins": [], "outs": [],
                "sync_info": {"on_wait": waits, "on_update": updates}}

    for f in m.get("functions", []):
        for blk in f.get("blocks", []):
            out = []
            for inst in blk.get("instructions", []):
                si = inst.get("sync_info")
                if not si:
                    out.append(inst)
                    continue
                waits = si.get("on_wait") or []
                ups = si.get("on_update") or []
                post = []
                if len(waits) > 1:
                    for w in waits[:-1]:
                        out.append(nop(inst["engine"], [w], []))
                    si["on_wait"] = [waits[-1]]
                if len(ups) > 1:
                    assert inst.get("opcode") not in (
                        "DMACopy", "DMATranspose", "DMAGather",
                        "DMAScatterAdd", "TriggerDma"), inst.get("name")
                    si["on_update"] = [ups[0]]
                    for u in ups[1:]:
                        post.append(nop(inst["engine"], [], [u]))
                out.append(inst)
                out.extend(post)
            blk["instructions"] = out
    return json.dumps(m).encode()


def _install_waitfix():
    import concourse.bass_utils as bu
    if getattr(bu, "_bitattn_waitfix", False):
        return
    bu._bitattn_waitfix = True
    orig = bu.compile_bir_kernel

    def patched(bir_json, tmpdir, neff_name="file.neff"):
        return orig(_legalize_bir_json(bir_json), tmpdir, neff_name)

    bu.compile_bir_kernel = patched
    try:
        import concourse.bass2jax as b2j
        if getattr(b2j, "compile_bir_kernel", None) is orig:
            b2j.compile_bir_kernel = patched
    except ImportError:
        pass


# ---------------------------------------------------------------- bass build
def _build_nc():
    import concourse.bass as bass
    import concourse.mybir as mybir
    import concourse.tile as tile
    import concourse.bass_isa as bass_isa
    from concourse import library_config
    from contextlib import ExitStack

    F32 = mybir.dt.float32
    BF = mybir.dt.bfloat16
    AF = mybir.ActivationFunctionType
    ALU = mybir.AluOpType
    RED = bass_isa.ReduceOp

    nc = bass.Bass(name="bitattn2", trn_type="TRN2")
    xT_in = nc.dram_tensor("xT", [D, T], BF, kind="ExternalInput")
    wqT_in = nc.dram_tensor("wqT", [D, OSH], F32, kind="ExternalInput")
    wkT_in = nc.dram_tensor("wkT", [D, OSH], F32, kind="ExternalInput")
    wvT_in = nc.dram_tensor("wvT", [D, OSH], F32, kind="ExternalInput")
    woT_in = nc.dram_tensor("woT", [OSH, D], F32, kind="ExternalInput")
    ropeC_in = nc.dram_tensor("ropeC", [P, T], BF, kind="ExternalInput")
    ropeS_in = nc.dram_tensor("ropeS", [P, T], BF, kind="ExternalInput")
    tri_in = nc.dram_tensor("tri", [P, P], BF, kind="ExternalInput")
    outT = nc.dram_tensor("outT", [D, T], BF, kind="ExternalOutput")

    xT_v = xT_in[:].rearrange("(g p) t -> g p t", p=P)
    w_views = {
        "q": wqT_in[:].rearrange("(g p) o -> g p o", p=P),
        "k": wkT_in[:].rearrange("(g p) o -> g p o", p=P),
        "v": wvT_in[:].rearrange("(g p) o -> g p o", p=P),
    }
    woT_v = woT_in[:].rearrange("(fc p) o -> fc p o", p=P)
    outT_v = outT[:].rearrange("(ob p) t -> ob p t", p=P)

    with tile.TileContext(nc) as tc, ExitStack() as ctx:
        nc.gpsimd.load_library(library_config.attn)

        glob = ctx.enter_context(tc.tile_pool(name="glob", bufs=1))
        ones_f = glob.tile([P, 1], F32)
        nc.gpsimd.memset(ones_f[:], 1.0)
        ones_b = glob.tile([P, 1], BF)
        nc.vector.tensor_copy(ones_b[:], ones_f[:])
        tri = glob.tile([P, P], BF)
        ropeC = glob.tile([P, T], BF)
        ropeS = glob.tile([P, T], BF)

        # ---------------- dequant mega-chunk helper --------------------
        # For MEGA groups side by side: [128, MEGA*OSH].  Per column o of
        # group g: s = sum_p(2|w|)/256; keep if 2|w| > s; W_dq = sign(w)*s.
        # partition_all_reduce keeps per-column sums independent, so two
        # groups can share one op along the free dim.
        dq_pend = []

        def dequant_front(dq, src_aps, dst_ap, fw):
            wt = dq.tile([P, fw], F32, tag="wt", name="wt")
            step = fw // len(src_aps)
            for i, ap in enumerate(src_aps):
                nc.sync.dma_start(wt[:, i * step:(i + 1) * step], ap)
            # abs scaled 1/128 so the f32 all-reduce is exactly s=mean|w|;
            # compare flips to ab2*256 > sbr (exact powers of 2).  Second
            # all-reduce in bf16 feeds the smooth scale multiplies in 2x.
            ab2 = dq.tile([P, fw], F32, tag="ab2", name="ab2")
            nc.scalar.activation(ab2[:], wt[:], AF.Abs, scale=1.0 / 128.0)
            sbr = dq.tile([P, fw], F32, tag="sbr", name="sbr")
            nc.gpsimd.partition_all_reduce(sbr[:], ab2[:], channels=P,
                                           reduce_op=RED.add)
            sbb = dq.tile([P, fw], BF, tag="sbb", name="sbb")
            nc.gpsimd.partition_all_reduce(sbb[:], ab2[:], channels=P,
                                           reduce_op=RED.add)
            m01 = dq.tile([P, fw], BF, tag="m01", name="m01")
            nc.vector.scalar_tensor_tensor(m01[:], ab2[:], 256.0,
                                           sbr[:], ALU.mult, ALU.is_gt)
            sgn = dq.tile([P, fw], BF, tag="sgn", name="sgn")
            nc.scalar.activation(sgn[:], wt[:], AF.Sign)
            dq_pend.append((dq, sbb, m01, sgn, dst_ap, fw))

        dq_n = [0]

        def dequant_back():
            dq, sbb, m01, sgn, dst_ap, fw = dq_pend.pop(0)
            ms = dq.tile([P, fw], BF, tag="ms", name="ms")
            nc.vector.tensor_tensor(ms[:], sbb[:], m01[:], ALU.mult)
            nc.vector.tensor_tensor(dst_ap, ms[:], sgn[:], ALU.mult)

        # emit back-half one chunk late: DVE never queues a mul that waits
        # on the same chunk's Pool ms (head-of-line stall)
        def dequant_mega(dq, src_aps, dst_ap, fw):
            dequant_front(dq, src_aps, dst_ap, fw)
            if len(dq_pend) > 1:
                dequant_back()

        # attention operands built in SBUF during phase A — no DRAM round trip
        apool = ctx.enter_context(tc.tile_pool(name="apool", bufs=1))
        khs = apool.tile([P, NH, T], BF, name="khs")
        vhs = apool.tile([P, G, NH * HD], BF, name="vhs")   # [keys, kb, 4h*d]
        y_sb = apool.tile([P, NH, T], BF, name="y_sb")
        qj_all = apool.tile([P, NTB, NH, TB], BF, name="qj_all")

        wopool = ctx.enter_context(tc.tile_pool(name="wop", bufs=1))
        wo_dq = wopool.tile([P, NH, D], BF, name="wo_dq")

        # ================= phase A: dequant + QKV + rope ===============
        with ExitStack() as pctx:
            wpool = pctx.enter_context(tc.tile_pool(name="wdqp", bufs=1))
            w_dq = {
                "q": wpool.tile([P, G, OSH], BF, name="wq_dq"),
                "k": wpool.tile([P, G, OSH], BF, name="wk_dq"),
                "v": wpool.tile([P, G, OSH], BF, name="wv_dq"),
            }
            dq = pctx.enter_context(tc.tile_pool(name="dq", bufs=3))
            first = [True]

            def dequant_all(pr):
                for m in range(G // MEGA):
                    gs = [m * MEGA + i for i in range(MEGA)]
                    dequant_mega(
                        dq, [w_views[pr][g] for g in gs],
                        w_dq[pr][:, gs[0]:gs[-1] + 1].rearrange(
                            "p g o -> p (g o)"),
                        MEGA * OSH)
                    if first[0]:
                        first[0] = False
                        nc.sync.dma_start(tri[:], tri_in[:])
                        nc.sync.dma_start(ropeC[:], ropeC_in[:])
                        nc.sync.dma_start(ropeS[:], ropeS_in[:])

            for pr in ("q", "k", "v"):
                dequant_all(pr)
            while dq_pend:
                dequant_back()

            # wo dequant chunks, spread across the tb sections below so the
            # DVE back-half never bunches against rope evac work
            wo_chunks = [(fc, m * MEGA * OSH) for fc in range(NH)
                         for m in range(4 // MEGA)]

            def emit_wo_chunks(n):
                for _ in range(n):
                    if not wo_chunks:
                        break
                    fc, oc0 = wo_chunks.pop(0)
                    dequant_mega(
                        dq,
                        [woT_v[fc, :, oc0 + i * OSH:oc0 + (i + 1) * OSH]
                         for i in range(MEGA)],
                        wo_dq[:, fc, oc0:oc0 + MEGA * OSH], MEGA * OSH)
                if not wo_chunks:
                    while dq_pend:
                        dequant_back()

            xpool = pctx.enter_context(tc.tile_pool(name="xp", bufs=2))
            evac = pctx.enter_context(tc.tile_pool(name="evac", bufs=2))
            psQK = pctx.enter_context(
                tc.tile_pool(name="psQK", bufs=1, space="PSUM"))
            psV = pctx.enter_context(
                tc.tile_pool(name="psV", bufs=2, space="PSUM"))

            for tb in range(NTB):
                ts = slice(tb * TB, (tb + 1) * TB)
                xTr = xpool.tile([P, G, TB], BF, tag="xTr", name="xTr")
                for g in range(G):
                    nc.sync.dma_start(xTr[:, g], xT_v[g, :, ts])

                # q, k projections: g-outer / head-inner so PE consumption
                # paces the dequant stream on tb=0.
                for pr in ("q", "k"):
                    pqs = [psQK.tile([P, TB], F32, tag=f"pq{h}", name=f"pq{h}")
                           for h in range(NH)]
                    for g in range(G):
                        for h in range(NH):
                            nc.tensor.matmul(
                                pqs[h][:], w_dq[pr][:, g, h * HD:(h + 1) * HD],
                                xTr[:, g], start=(g == 0), stop=(g == G - 1))
                    for h in range(NH):
                        qe = evac.tile([P, TB], BF, tag="qe", name="qe")
                        nc.scalar.copy(qe[:], pqs[h][:])
                        qa = evac.tile([P, TB], BF, tag="qa", name="qa")
                        nc.vector.tensor_tensor(qa[:], qe[:],
                                                ropeC[:, ts], ALU.mult)
                        qs = evac.tile([P, TB], BF, tag="qs", name="qs")
                        nc.vector.tensor_tensor(qs[:], qe[:],
                                                ropeS[:, ts], ALU.mult)
                        # rope half-swap: SBUF->SBUF DMA (engines can't cross
                        # partition offsets), then one same-partition add
                        tsw = evac.tile([P, TB], BF, tag="tsw", name="tsw")
                        nc.sync.dma_start(tsw[0:64], qs[64:128])
                        nc.sync.dma_start(tsw[64:128], qs[0:64])
                        if pr == "q":
                            dst = qj_all[:, tb, h]
                        else:
                            dst = khs[:, h, tb * TB:(tb + 1) * TB]
                        nc.vector.tensor_tensor(dst, qa[:], tsw[:], ALU.add)

                # v projection, output directly [keys, kb, vdims] in SBUF
                for tk in range(NTB):
                    pv = psV.tile([P, TB], F32, tag="pv", name="pv")
                    for g in range(G):
                        nc.tensor.matmul(
                            pv[:], xTr[:, g, tk * HD:(tk + 1) * HD],
                            w_dq["v"][:, g], start=(g == 0), stop=(g == G - 1))
                    nc.scalar.copy(vhs[:, tb * NTB + tk], pv[:])

                if tb >= 1:
                    emit_wo_chunks(6)

        # ============ phase B: attention + o_proj ======================
        with ExitStack() as pctx:
            expool = pctx.enter_context(tc.tile_pool(name="exp", bufs=6))
            sspool = pctx.enter_context(tc.tile_pool(name="ss", bufs=3))
            opool = pctx.enter_context(tc.tile_pool(name="op", bufs=4))
            psS = pctx.enter_context(
                tc.tile_pool(name="psS", bufs=4, space="PSUM"))
            psY = pctx.enter_context(
                tc.tile_pool(name="psY", bufs=2, space="PSUM"))
            psD = pctx.enter_context(
                tc.tile_pool(name="psD", bufs=1, space="PSUM"))
            psO = pctx.enter_context(
                tc.tile_pool(name="psO", bufs=1, space="PSUM"))

            def oproj_blocks(tb, obs):
                ts = slice(tb * TB, (tb + 1) * TB)
                for ob in obs:
                    ps_o = psO.tile([P, TB], F32, tag="ps_o", name="ps_o")
                    for fc in range(NH):
                        nc.tensor.matmul(
                            ps_o[:], wo_dq[:, fc, ob * P:(ob + 1) * P],
                            y_sb[:, fc, ts],
                            start=(fc == 0), stop=(fc == NH - 1))
                    ot = opool.tile([P, TB], BF, tag="ot", name="ot")
                    if ob % 2 == 0:
                        nc.scalar.copy(ot[:], ps_o[:])
                    else:
                        nc.vector.tensor_copy(ot[:], ps_o[:])
                    nc.sync.dma_start(outT_v[ob, :, ts], ot[:])

            for j in range(NTB):
                ts = slice(j * TB, (j + 1) * TB)
                nkk = 4 * j + 4
                for h in range(NH):
                    qj = qj_all[:, j, h]
                    ps_y = psY.tile([P, TB], F32, tag="py", name="py")
                    exs = sspool.tile([P, TB], BF, tag="exs", name="exs")
                    pend = []  # (kk, ex, off, ncols)

                    def flush_pv(kk, ex, off, ncols):
                        nc.tensor.matmul(ps_y[:, off:],
                                         vhs[:, kk, h * HD:(h + 1) * HD],
                                         ex[:, 0:ncols],
                                         start=(kk == 0), stop=(kk == nkk - 1))

                    for kk in range(nkk):
                        d = kk - 4 * j
                        off = P * d if d >= 0 else 0
                        ncols = TB - off
                        ps_st = psS.tile([P, TB], F32, tag="st", name="st")
                        st = ps_st[:, 0:ncols]
                        nc.tensor.matmul(
                            st, khs[:, h, kk * P:(kk + 1) * P],
                            qj[:, off:TB], start=True, stop=True)
                        if d >= 0:
                            nc.vector.tensor_tensor(ps_st[:, 0:P],
                                                    ps_st[:, 0:P], tri[:],
                                                    ALU.add)
                        ex = expool.tile([P, TB], BF, tag="ex", name="ex")
                        nc.scalar.activation(ex[:, 0:ncols], st, AF.Exp,
                                             scale=SCALE)
                        if kk == 0:
                            nc.vector.tensor_copy(exs[:], ex[:])
                        else:
                            nc.vector.tensor_tensor(
                                exs[:, off:], exs[:, off:], ex[:, 0:ncols],
                                ALU.add)
                        pend.append((kk, ex, off, ncols))
                        if len(pend) > 3:
                            flush_pv(*pend.pop(0))
                    while pend:
                        flush_pv(*pend.pop(0))

                    ps_den = psD.tile([1, TB], F32, tag="pd", name="pd")
                    nc.tensor.matmul(ps_den[:], ones_b[:], exs[:],
                                     start=True, stop=True)
                    rec = expool.tile([1, TB], BF, tag="rec", name="rec")
                    with nc.allow_low_precision("bf16 1/denom"):
                        nc.vector.reciprocal(rec[:], ps_den[:])
                    den_b = sspool.tile([P, TB], BF, tag="den_b",
                                        name="den_b")
                    nc.gpsimd.partition_broadcast(den_b[:], rec[:])
                    nc.vector.tensor_tensor(y_sb[:, h, ts], ps_y[:],
                                            den_b[:], ALU.mult)
                oproj_blocks(j, range(G))

    from concourse import library_overlay
    library_overlay.lower_extended_insts(nc)
    return nc


def _rope_tables():
    half = HD // 2
    inv_freq = 1.0 / (10000.0 ** (np.arange(half, dtype=np.float64) / half))
    freqs = np.outer(np.arange(T, dtype=np.float64), inv_freq)  # [T, 64]
    c = np.cos(freqs).T  # [64, T]
    s = np.sin(freqs).T
    # S'' = [sin; -sin]: rope = q*C + swap_halves(q*S'')
    return (np.concatenate([c, c], axis=0),
            np.concatenate([s, -s], axis=0))


def kernel(x, w_q, w_k, w_v, w_o):
    _install_waitfix()
    from concourse.bass_utils import run_bass_kernel_spmd
    import ml_dtypes

    bf16 = ml_dtypes.bfloat16
    x = np.asarray(x, dtype=np.float32)
    w_q = np.asarray(w_q, dtype=np.float32)
    w_k = np.asarray(w_k, dtype=np.float32)
    w_v = np.asarray(w_v, dtype=np.float32)
    w_o = np.asarray(w_o, dtype=np.float32)
    B = x.shape[0]

    if "nc" not in _cached:
        _cached["nc"] = _build_nc()
    nc = _cached["nc"]

    ropeC, ropeS = _rope_tables()
    idx = np.arange(P)
    tri = np.where(idx[:, None] > idx[None, :], np.float32(NEG),
                   np.float32(0.0))

    in_maps = []
    for c in range(8):
        b, hg = divmod(c, 4)
        osl = slice(hg * OSH, (hg + 1) * OSH)
        in_maps.append({
            "xT": np.ascontiguousarray(x[b].T).astype(bf16),
            "wqT": np.ascontiguousarray(w_q[osl, :].T),
            "wkT": np.ascontiguousarray(w_k[osl, :].T),
            "wvT": np.ascontiguousarray(w_v[osl, :].T),
            "woT": np.ascontiguousarray(w_o[:, osl].T),
            "ropeC": ropeC.astype(bf16), "ropeS": ropeS.astype(bf16),
            "tri": tri.astype(bf16),
        })

    import os as _os
    trace = _os.environ.get("BITATTN_TRACE") == "1"
    res = run_bass_kernel_spmd(nc, in_maps, core_ids=list(range(8)),
                               trace=trace)
    _cached["last_res"] = res
    out = np.zeros((B, T, D), dtype=np.float32)
    for c in range(8):
        b = c // 4
        out[b] += res.results[c]["outT"].astype(np.float32).T
    return out


# revision 9
# speedup vs baseline: 63.6357x; 1.0116x over previous
"""BitSelfAttention TRN2 kernel v2 (8 NeuronCores, TP over heads + batch over B).

Core c -> batch b=c//4, head group hg=c%4 (heads 4*hg..4*hg+3).
All matmuls bf16 (1 cy/row); dequant threshold math kept f32-exact:
scale sum via gpsimd partition_all_reduce (f32), compare via DVE STT.
Rope half-swap baked into the DRAM store (swapped row ranges); reload adds
qa + swap(qs) on DVE in bf16.  Softmax denominator via DVE-accumulated exp
sums + one [1,512] matmul per (j,h).  o_proj PSUM evac on Pool engine.
Self-contained; includes the walrus one-wait BIR legalizer.
"""
import json
import numpy as np

# ---------------------------------------------------------------- constants
P = 128
T = 2048
D = 2048
NH = 4                     # heads per core
HD = 128                   # head dim
TB = 512                   # t-block
NTB = T // TB              # 4
G = D // P                 # 16 contraction chunks
OSH = 512                  # per-core qkv output-column shard
SCALE = HD ** -0.5
NEG = -1e30
MEGA = 1                   # dequant chunks merged per op

_cached = {}


# ------------------------------------------------------------- BIR legalizer
def _legalize_bir_json(bir_json: bytes) -> bytes:
    """This walrus accepts only ONE sync-wait (and update) per instruction.
    Hoist extras onto same-engine NoOps (engine FIFO keeps semantics)."""
    m = json.loads(bir_json)
    n = [0]

    def nop(engine, waits, updates):
        n[0] += 1
        return {"name": f"I-wfix{n[0]}", "opcode": "NoOp", "engine": engine,
                "ins": [], "outs": [],
                "sync_info": {"on_wait": waits, "on_update": updates}}

    for f in m.get("functions", []):
        for blk in f.get("blocks", []):
            out = []
            for inst in blk.get("instructions", []):
                si = inst.get("sync_info")
                if not si:
                    out.append(inst)
                    continue
                waits = si.get("on_wait") or []
                ups = si.get("on_update") or []
                post = []
                if len(waits) > 1:
                    for w in waits[:-1]:
                        out.append(nop(inst["engine"], [w], []))
                    si["on_wait"] = [waits[-1]]
                if len(ups) > 1:
                    assert inst.get("opcode") not in (
                        "DMACopy", "DMATranspose", "DMAGather",
                        "DMAScatterAdd", "TriggerDma"), inst.get("name")
                    si["on_update"] = [ups[0]]
                    for u in ups[1:]:
                        post.append(nop(inst["engine"], [], [u]))
                out.append(inst)
                out.extend(post)
            blk["instructions"] = out
    return json.dumps(m).encode()


def _install_waitfix():
    import concourse.bass_utils as bu
    if getattr(bu, "_bitattn_waitfix", False):
        return
    bu._bitattn_waitfix = True
    orig = bu.compile_bir_kernel

    def patched(bir_json, tmpdir, neff_name="file.neff"):
        return orig(_legalize_bir_json(bir_json), tmpdir, neff_name)

    bu.compile_bir_kernel = patched
    try:
        import concourse.bass2jax as b2j
        if getattr(b2j, "compile_bir_kernel", None) is orig:
            b2j.compile_bir_kernel = patched
    except ImportError:
        pass


# ---------------------------------------------------------------- bass build
def _build_nc():
    import concourse.bass as bass
    import concourse.mybir as mybir
    import concourse.tile as tile
    import concourse.bass_isa as bass_isa
    from concourse import library_config
    from contextlib import ExitStack

    F32 = mybir.dt.float32
    BF = mybir.dt.bfloat16
    AF = mybir.ActivationFunctionType
    ALU = mybir.AluOpType
    RED = bass_isa.ReduceOp

    nc = bass.Bass(name="bitattn2", trn_type="TRN2")
    xT_in = nc.dram_tensor("xT", [D, T], BF, kind="ExternalInput")
    wqT_in = nc.dram_tensor("wqT", [D, OSH], F32, kind="ExternalInput")
    wkT_in = nc.dram_tensor("wkT", [D, OSH], F32, kind="ExternalInput")
    wvT_in = nc.dram_tensor("wvT", [D, OSH], F32, kind="ExternalInput")
    woT_in = nc.dram_tensor("woT", [OSH, D], F32, kind="ExternalInput")
    ropeC_in = nc.dram_tensor("ropeC", [P, T], BF, kind="ExternalInput")
    ropeS_in = nc.dram_tensor("ropeS", [P, T], BF, kind="ExternalInput")
    tri_in = nc.dram_tensor("tri", [P, P], BF, kind="ExternalInput")
    outT = nc.dram_tensor("outT", [D, T], BF, kind="ExternalOutput")

    xT_v = xT_in[:].rearrange("(g p) t -> g p t", p=P)
    w_views = {
        "q": wqT_in[:].rearrange("(g p) o -> g p o", p=P),
        "k": wkT_in[:].rearrange("(g p) o -> g p o", p=P),
        "v": wvT_in[:].rearrange("(g p) o -> g p o", p=P),
    }
    woT_v = woT_in[:].rearrange("(fc p) o -> fc p o", p=P)
    outT_v = outT[:].rearrange("(ob p) t -> ob p t", p=P)

    with tile.TileContext(nc) as tc, ExitStack() as ctx:
        nc.gpsimd.load_library(library_config.attn)

        glob = ctx.enter_context(tc.tile_pool(name="glob", bufs=1))
        ones_b = glob.tile([P, 1], BF)
        nc.gpsimd.memset(ones_b[:], 1.0)
        tri = glob.tile([P, P], BF)
        ropeC = glob.tile([P, T], BF)
        ropeS = glob.tile([P, T], BF)

        # ---------------- dequant mega-chunk helper --------------------
        # For MEGA groups side by side: [128, MEGA*OSH].  Per column o of
        # group g: s = sum_p(2|w|)/256; keep if 2|w| > s; W_dq = sign(w)*s.
        # partition_all_reduce keeps per-column sums independent, so two
        # groups can share one op along the free dim.
        dq_pend = []

        def dequant_front(dq, src_aps, dst_ap, fw):
            wt = dq.tile([P, fw], F32, tag="wt", name="wt")
            step = fw // len(src_aps)
            for i, ap in enumerate(src_aps):
                nc.sync.dma_start(wt[:, i * step:(i + 1) * step], ap)
            # abs scaled 1/128 so the f32 all-reduce is exactly s=mean|w|;
            # compare flips to ab2*256 > sbr (exact powers of 2).  Second
            # all-reduce in bf16 feeds the smooth scale multiplies in 2x.
            ab2 = dq.tile([P, fw], F32, tag="ab2", name="ab2")
            nc.scalar.activation(ab2[:], wt[:], AF.Abs, scale=1.0 / 128.0)
            sbr = dq.tile([P, fw], F32, tag="sbr", name="sbr")
            nc.gpsimd.partition_all_reduce(sbr[:], ab2[:], channels=P,
                                           reduce_op=RED.add)
            sbb = dq.tile([P, fw], BF, tag="sbb", name="sbb")
            nc.gpsimd.partition_all_reduce(sbb[:], ab2[:], channels=P,
                                           reduce_op=RED.add)
            m01 = dq.tile([P, fw], BF, tag="m01", name="m01")
            nc.vector.scalar_tensor_tensor(m01[:], ab2[:], 256.0,
                                           sbr[:], ALU.mult, ALU.is_gt)
            sgn = dq.tile([P, fw], BF, tag="sgn", name="sgn")
            nc.scalar.activation(sgn[:], wt[:], AF.Sign)
            dq_pend.append((dq, sbb, m01, sgn, dst_ap, fw))

        dq_n = [0]

        def dequant_back():
            dq, sbb, m01, sgn, dst_ap, fw = dq_pend.pop(0)
            ms = dq.tile([P, fw], BF, tag="m01", name="ms")
            nc.vector.tensor_tensor(ms[:], sbb[:], m01[:], ALU.mult)
            nc.vector.tensor_tensor(dst_ap, ms[:], sgn[:], ALU.mult)

        # emit back-half one chunk late: DVE never queues a mul that waits
        # on the same chunk's Pool ms (head-of-line stall)
        def dequant_mega(dq, src_aps, dst_ap, fw):
            dequant_front(dq, src_aps, dst_ap, fw)
            if len(dq_pend) > 1:
                dequant_back()

        # attention operands built in SBUF during phase A — no DRAM round trip
        apool = ctx.enter_context(tc.tile_pool(name="apool", bufs=1))
        khs = apool.tile([P, NH, T], BF, name="khs")
        vhs = apool.tile([P, G, NH * HD], BF, name="vhs")   # [keys, kb, 4h*d]
        y_sb = apool.tile([P, NH, T], BF, name="y_sb")
        qj_all = apool.tile([P, NTB, NH, TB], BF, name="qj_all")

        wopool = ctx.enter_context(tc.tile_pool(name="wop", bufs=1))
        wo_dq = wopool.tile([P, NH, D], BF, name="wo_dq")

        # ================= phase A: dequant + QKV + rope ===============
        with ExitStack() as pctx:
            wpool = pctx.enter_context(tc.tile_pool(name="wdqp", bufs=1))
            w_dq = {
                "q": wpool.tile([P, G, OSH], BF, name="wq_dq"),
                "k": wpool.tile([P, G, OSH], BF, name="wk_dq"),
                "v": wpool.tile([P, G, OSH], BF, name="wv_dq"),
            }
            dq = pctx.enter_context(tc.tile_pool(name="dq", bufs=3))
            first = [True]

            def dequant_all(pr):
                for m in range(G // MEGA):
                    gs = [m * MEGA + i for i in range(MEGA)]
                    dequant_mega(
                        dq, [w_views[pr][g] for g in gs],
                        w_dq[pr][:, gs[0]:gs[-1] + 1].rearrange(
                            "p g o -> p (g o)"),
                        MEGA * OSH)
                    if first[0]:
                        first[0] = False
                        nc.sync.dma_start(tri[:], tri_in[:])
                        nc.sync.dma_start(ropeC[:], ropeC_in[:])
                        nc.sync.dma_start(ropeS[:], ropeS_in[:])

            for pr in ("q", "k", "v"):
                dequant_all(pr)
            while dq_pend:
                dequant_back()

            # wo dequant chunks, spread across the tb sections below so the
            # DVE back-half never bunches against rope evac work
            wo_chunks = [(fc, m * MEGA * OSH) for fc in range(NH)
                         for m in range(4 // MEGA)]

            def emit_wo_chunks(n):
                for _ in range(n):
                    if not wo_chunks:
                        break
                    fc, oc0 = wo_chunks.pop(0)
                    dequant_mega(
                        dq,
                        [woT_v[fc, :, oc0 + i * OSH:oc0 + (i + 1) * OSH]
                         for i in range(MEGA)],
                        wo_dq[:, fc, oc0:oc0 + MEGA * OSH], MEGA * OSH)
                if not wo_chunks:
                    while dq_pend:
                        dequant_back()

            xpool = pctx.enter_context(tc.tile_pool(name="xp", bufs=2))
            evac = pctx.enter_context(tc.tile_pool(name="evac", bufs=3))
            psQK = pctx.enter_context(
                tc.tile_pool(name="psQK", bufs=1, space="PSUM"))
            psV = pctx.enter_context(
                tc.tile_pool(name="psV", bufs=2, space="PSUM"))

            for tb in range(NTB):
                ts = slice(tb * TB, (tb + 1) * TB)
                xTr = xpool.tile([P, G, TB], BF, tag="xTr", name="xTr")
                for g in range(G):
                    nc.sync.dma_start(xTr[:, g], xT_v[g, :, ts])

                # q, k projections: g-outer / head-inner so PE consumption
                # paces the dequant stream on tb=0.
                for pr in ("q", "k"):
                    pqs = [psQK.tile([P, TB], F32, tag=f"pq{h}", name=f"pq{h}")
                           for h in range(NH)]
                    for g in range(G):
                        for h in range(NH):
                            nc.tensor.matmul(
                                pqs[h][:], w_dq[pr][:, g, h * HD:(h + 1) * HD],
                                xTr[:, g], start=(g == 0), stop=(g == G - 1))
                    for h in range(NH):
                        qe = evac.tile([P, TB], BF, tag="qe", name="qe")
                        nc.scalar.copy(qe[:], pqs[h][:])
                        qa = evac.tile([P, TB], BF, tag="qa", name="qa")
                        nc.vector.tensor_tensor(qa[:], qe[:],
                                                ropeC[:, ts], ALU.mult)
                        qs = evac.tile([P, TB], BF, tag="qs", name="qs")
                        nc.vector.tensor_tensor(qs[:], qe[:],
                                                ropeS[:, ts], ALU.mult)
                        # rope half-swap: SBUF->SBUF DMA (engines can't cross
                        # partition offsets), then one same-partition add
                        tsw = evac.tile([P, TB], BF, tag="tsw", name="tsw")
                        nc.sync.dma_start(tsw[0:64], qs[64:128])
                        nc.sync.dma_start(tsw[64:128], qs[0:64])
                        if pr == "q":
                            dst = qj_all[:, tb, h]
                        else:
                            dst = khs[:, h, tb * TB:(tb + 1) * TB]
                        nc.vector.tensor_tensor(dst, qa[:], tsw[:], ALU.add)

                # v projection, output directly [keys, kb, vdims] in SBUF
                for tk in range(NTB):
                    pv = psV.tile([P, TB], F32, tag="pv", name="pv")
                    for g in range(G):
                        nc.tensor.matmul(
                            pv[:], xTr[:, g, tk * HD:(tk + 1) * HD],
                            w_dq["v"][:, g], start=(g == 0), stop=(g == G - 1))
                    nc.scalar.copy(vhs[:, tb * NTB + tk], pv[:])

                if tb >= 1:
                    emit_wo_chunks(6)

        # ============ phase B: attention + o_proj ======================
        with ExitStack() as pctx:
            expool = pctx.enter_context(tc.tile_pool(name="exp", bufs=6))
            sspool = pctx.enter_context(tc.tile_pool(name="ss", bufs=4))
            opool = pctx.enter_context(tc.tile_pool(name="op", bufs=4))
            psS = pctx.enter_context(
                tc.tile_pool(name="psS", bufs=4, space="PSUM"))
            psY = pctx.enter_context(
                tc.tile_pool(name="psY", bufs=2, space="PSUM"))
            psD = pctx.enter_context(
                tc.tile_pool(name="psD", bufs=1, space="PSUM"))
            psO = pctx.enter_context(
                tc.tile_pool(name="psO", bufs=1, space="PSUM"))

            def oproj_blocks(tb, obs):
                ts = slice(tb * TB, (tb + 1) * TB)
                for ob in obs:
                    ps_o = psO.tile([P, TB], F32, tag="ps_o", name="ps_o")
                    for fc in range(NH):
                        nc.tensor.matmul(
                            ps_o[:], wo_dq[:, fc, ob * P:(ob + 1) * P],
                            y_sb[:, fc, ts],
                            start=(fc == 0), stop=(fc == NH - 1))
                    ot = opool.tile([P, TB], BF, tag="ot", name="ot")
                    if ob % 2 == 0:
                        nc.scalar.copy(ot[:], ps_o[:])
                    else:
                        nc.vector.tensor_copy(ot[:], ps_o[:])
                    nc.sync.dma_start(outT_v[ob, :, ts], ot[:])

            for j in range(NTB):
                ts = slice(j * TB, (j + 1) * TB)
                nkk = 4 * j + 4
                for h in range(NH):
                    qj = qj_all[:, j, h]
                    ps_y = psY.tile([P, TB], F32, tag="py", name="py")
                    exs = sspool.tile([P, TB], BF, tag="exs", name="exs")
                    pend = []  # (kk, ex, off, ncols)

                    def flush_pv(kk, ex, off, ncols):
                        nc.tensor.matmul(ps_y[:, off:],
                                         vhs[:, kk, h * HD:(h + 1) * HD],
                                         ex[:, 0:ncols],
                                         start=(kk == 0), stop=(kk == nkk - 1))

                    for kk in range(nkk):
                        d = kk - 4 * j
                        off = P * d if d >= 0 else 0
                        ncols = TB - off
                        ps_st = psS.tile([P, TB], F32, tag="st", name="st")
                        st = ps_st[:, 0:ncols]
                        nc.tensor.matmul(
                            st, khs[:, h, kk * P:(kk + 1) * P],
                            qj[:, off:TB], start=True, stop=True)
                        if d >= 0:
                            nc.vector.tensor_tensor(ps_st[:, 0:P],
                                                    ps_st[:, 0:P], tri[:],
                                                    ALU.add)
                        ex = expool.tile([P, TB], BF, tag="ex", name="ex")
                        nc.scalar.activation(ex[:, 0:ncols], st, AF.Exp,
                                             scale=SCALE)
                        if kk == 0:
                            nc.vector.tensor_copy(exs[:], ex[:])
                        else:
                            nc.vector.tensor_tensor(
                                exs[:, off:], exs[:, off:], ex[:, 0:ncols],
                                ALU.add)
                        pend.append((kk, ex, off, ncols))
                        if len(pend) > 3:
                            flush_pv(*pend.pop(0))
                    while pend:
                        flush_pv(*pend.pop(0))

                    ps_den = psD.tile([1, TB], F32, tag="pd", name="pd")
                    nc.tensor.matmul(ps_den[:], ones_b[:], exs[:],
                                     start=True, stop=True)
                    rec = expool.tile([1, TB], BF, tag="rec", name="rec")
                    with nc.allow_low_precision("bf16 1/denom"):
                        nc.vector.reciprocal(rec[:], ps_den[:])
                    den_b = sspool.tile([P, TB], BF, tag="den_b",
                                        name="den_b")
                    nc.gpsimd.partition_broadcast(den_b[:], rec[:])
                    nc.vector.tensor_tensor(y_sb[:, h, ts], ps_y[:],
                                            den_b[:], ALU.mult)
                oproj_blocks(j, range(G))

    from concourse import library_overlay
    library_overlay.lower_extended_insts(nc)
    return nc


def _rope_tables():
    half = HD // 2
    inv_freq = 1.0 / (10000.0 ** (np.arange(half, dtype=np.float64) / half))
    freqs = np.outer(np.arange(T, dtype=np.float64), inv_freq)  # [T, 64]
    c = np.cos(freqs).T  # [64, T]
    s = np.sin(freqs).T
    # S'' = [sin; -sin]: rope = q*C + swap_halves(q*S'')
    return (np.concatenate([c, c], axis=0),
            np.concatenate([s, -s], axis=0))


def kernel(x, w_q, w_k, w_v, w_o):
    _install_waitfix()
    from concourse.bass_utils import run_bass_kernel_spmd
    import ml_dtypes

    bf16 = ml_dtypes.bfloat16
    x = np.asarray(x, dtype=np.float32)
    w_q = np.asarray(w_q, dtype=np.float32)
    w_k = np.asarray(w_k, dtype=np.float32)
    w_v = np.asarray(w_v, dtype=np.float32)
    w_o = np.asarray(w_o, dtype=np.float32)
    B = x.shape[0]

    if "nc" not in _cached:
        _cached["nc"] = _build_nc()
    nc = _cached["nc"]

    ropeC, ropeS = _rope_tables()
    idx = np.arange(P)
    tri = np.where(idx[:, None] > idx[None, :], np.float32(NEG),
                   np.float32(0.0))

    in_maps = []
    for c in range(8):
        b, hg = divmod(c, 4)
        osl = slice(hg * OSH, (hg + 1) * OSH)
        in_maps.append({
            "xT": np.ascontiguousarray(x[b].T).astype(bf16),
            "wqT": np.ascontiguousarray(w_q[osl, :].T),
            "wkT": np.ascontiguousarray(w_k[osl, :].T),
            "wvT": np.ascontiguousarray(w_v[osl, :].T),
            "woT": np.ascontiguousarray(w_o[:, osl].T),
            "ropeC": ropeC.astype(bf16), "ropeS": ropeS.astype(bf16),
            "tri": tri.astype(bf16),
        })

    import os as _os
    trace = _os.environ.get("BITATTN_TRACE") == "1"
    res = run_bass_kernel_spmd(nc, in_maps, core_ids=list(range(8)),
                               trace=trace)
    _cached["last_res"] = res
    out = np.zeros((B, T, D), dtype=np.float32)
    for c in range(8):
        b = c // 4
        out[b] += res.results[c]["outT"].astype(np.float32).T
    return out


# revision 10
# speedup vs baseline: 64.5842x; 1.0149x over previous
"""BitSelfAttention TRN2 kernel v2 (8 NeuronCores, TP over heads + batch over B).

Core c -> batch b=c//4, head group hg=c%4 (heads 4*hg..4*hg+3).
All matmuls bf16 (1 cy/row); dequant threshold math kept f32-exact:
scale sum via gpsimd partition_all_reduce (f32), compare via DVE STT.
Rope half-swap baked into the DRAM store (swapped row ranges); reload adds
qa + swap(qs) on DVE in bf16.  Softmax denominator via DVE-accumulated exp
sums + one [1,512] matmul per (j,h).  o_proj PSUM evac on Pool engine.
Self-contained; includes the walrus one-wait BIR legalizer.
"""
import json
import numpy as np

# ---------------------------------------------------------------- constants
P = 128
T = 2048
D = 2048
NH = 4                     # heads per core
HD = 128                   # head dim
TB = 512                   # t-block
NTB = T // TB              # 4
G = D // P                 # 16 contraction chunks
OSH = 512                  # per-core qkv output-column shard
SCALE = HD ** -0.5
NEG = -1e30
MEGA = 1                   # dequant chunks merged per op

_cached = {}


# ------------------------------------------------------------- BIR legalizer
def _legalize_bir_json(bir_json: bytes) -> bytes:
    """This walrus accepts only ONE sync-wait (and update) per instruction.
    Hoist extras onto same-engine NoOps (engine FIFO keeps semantics)."""
    m = json.loads(bir_json)
    n = [0]

    def nop(engine, waits, updates):
        n[0] += 1
        return {"name": f"I-wfix{n[0]}", "opcode": "NoOp", "engine": engine,
                "ins": [], "outs": [],
                "sync_info": {"on_wait": waits, "on_update": updates}}

    for f in m.get("functions", []):
        for blk in f.get("blocks", []):
            out = []
            for inst in blk.get("instructions", []):
                si = inst.get("sync_info")
                if not si:
                    out.append(inst)
                    continue
                waits = si.get("on_wait") or []
                ups = si.get("on_update") or []
                post = []
                if len(waits) > 1:
                    for w in waits[:-1]:
                        out.append(nop(inst["engine"], [w], []))
                    si["on_wait"] = [waits[-1]]
                if len(ups) > 1:
                    assert inst.get("opcode") not in (
                        "DMACopy", "DMATranspose", "DMAGather",
                        "DMAScatterAdd", "TriggerDma"), inst.get("name")
                    si["on_update"] = [ups[0]]
                    for u in ups[1:]:
                        post.append(nop(inst["engine"], [], [u]))
                out.append(inst)
                out.extend(post)
            blk["instructions"] = out
    return json.dumps(m).encode()


def _install_waitfix():
    import concourse.bass_utils as bu
    if getattr(bu, "_bitattn_waitfix", False):
        return
    bu._bitattn_waitfix = True
    orig = bu.compile_bir_kernel

    def patched(bir_json, tmpdir, neff_name="file.neff"):
        return orig(_legalize_bir_json(bir_json), tmpdir, neff_name)

    bu.compile_bir_kernel = patched
    try:
        import concourse.bass2jax as b2j
        if getattr(b2j, "compile_bir_kernel", None) is orig:
            b2j.compile_bir_kernel = patched
    except ImportError:
        pass


# ---------------------------------------------------------------- bass build
def _build_nc():
    import concourse.bass as bass
    import concourse.mybir as mybir
    import concourse.tile as tile
    import concourse.bass_isa as bass_isa
    from concourse import library_config
    from contextlib import ExitStack

    F32 = mybir.dt.float32
    BF = mybir.dt.bfloat16
    AF = mybir.ActivationFunctionType
    ALU = mybir.AluOpType
    RED = bass_isa.ReduceOp

    nc = bass.Bass(name="bitattn2", trn_type="TRN2")
    xT_in = nc.dram_tensor("xT", [D, T], BF, kind="ExternalInput")
    wqT_in = nc.dram_tensor("wqT", [D, OSH], F32, kind="ExternalInput")
    wkT_in = nc.dram_tensor("wkT", [D, OSH], F32, kind="ExternalInput")
    wvT_in = nc.dram_tensor("wvT", [D, OSH], F32, kind="ExternalInput")
    woT_in = nc.dram_tensor("woT", [OSH, D], F32, kind="ExternalInput")
    ropeC_in = nc.dram_tensor("ropeC", [P, T], BF, kind="ExternalInput")
    ropeS_in = nc.dram_tensor("ropeS", [P, T], BF, kind="ExternalInput")
    tri_in = nc.dram_tensor("tri", [P, P], BF, kind="ExternalInput")
    outT = nc.dram_tensor("outT", [D, T], BF, kind="ExternalOutput")

    xT_v = xT_in[:].rearrange("(g p) t -> g p t", p=P)
    w_views = {
        "q": wqT_in[:].rearrange("(g p) o -> g p o", p=P),
        "k": wkT_in[:].rearrange("(g p) o -> g p o", p=P),
        "v": wvT_in[:].rearrange("(g p) o -> g p o", p=P),
    }
    woT_v = woT_in[:].rearrange("(fc p) o -> fc p o", p=P)
    outT_v = outT[:].rearrange("(ob p) t -> ob p t", p=P)

    with tile.TileContext(nc) as tc, ExitStack() as ctx:
        nc.gpsimd.load_library(library_config.attn)

        glob = ctx.enter_context(tc.tile_pool(name="glob", bufs=1))
        ones_b = glob.tile([P, 1], BF)
        nc.gpsimd.memset(ones_b[:], 1.0)
        tri = glob.tile([P, P], BF)
        ropeC = glob.tile([P, T], BF)
        ropeS = glob.tile([P, T], BF)

        # ---------------- dequant mega-chunk helper --------------------
        # For MEGA groups side by side: [128, MEGA*OSH].  Per column o of
        # group g: s = sum_p(2|w|)/256; keep if 2|w| > s; W_dq = sign(w)*s.
        # partition_all_reduce keeps per-column sums independent, so two
        # groups can share one op along the free dim.
        dq_pend = []

        def dequant_front(dq, src_aps, dst_ap, fw):
            wt = dq.tile([P, fw], F32, tag="wt", name="wt")
            step = fw // len(src_aps)
            for i, ap in enumerate(src_aps):
                nc.sync.dma_start(wt[:, i * step:(i + 1) * step], ap)
            # abs scaled 1/128 so the f32 all-reduce is exactly s=mean|w|;
            # compare flips to ab2*256 > sbr (exact powers of 2).  Second
            # all-reduce in bf16 feeds the smooth scale multiplies in 2x.
            ab2 = dq.tile([P, fw], F32, tag="ab2", name="ab2")
            nc.scalar.activation(ab2[:], wt[:], AF.Abs, scale=1.0 / 128.0)
            sbr = dq.tile([P, fw], F32, tag="sbr", name="sbr")
            nc.gpsimd.partition_all_reduce(sbr[:], ab2[:], channels=P,
                                           reduce_op=RED.add)
            sbb = dq.tile([P, fw], BF, tag="sbb", name="sbb")
            nc.gpsimd.partition_all_reduce(sbb[:], ab2[:], channels=P,
                                           reduce_op=RED.add)
            m01 = dq.tile([P, fw], BF, tag="m01", name="m01")
            nc.vector.scalar_tensor_tensor(m01[:], ab2[:], 256.0,
                                           sbr[:], ALU.mult, ALU.is_gt)
            sgn = dq.tile([P, fw], BF, tag="sgn", name="sgn")
            nc.scalar.activation(sgn[:], wt[:], AF.Sign)
            dq_pend.append((dq, sbb, m01, sgn, dst_ap, fw))

        dq_n = [0]

        def dequant_back():
            dq, sbb, m01, sgn, dst_ap, fw = dq_pend.pop(0)
            ms = dq.tile([P, fw], BF, tag="m01", name="ms")
            nc.vector.tensor_tensor(ms[:], sbb[:], m01[:], ALU.mult)
            nc.vector.tensor_tensor(dst_ap, ms[:], sgn[:], ALU.mult)

        # emit back-half one chunk late: DVE never queues a mul that waits
        # on the same chunk's Pool ms (head-of-line stall)
        def dequant_mega(dq, src_aps, dst_ap, fw):
            dequant_front(dq, src_aps, dst_ap, fw)
            if len(dq_pend) > 1:
                dequant_back()

        # attention operands built in SBUF during phase A — no DRAM round trip
        apool = ctx.enter_context(tc.tile_pool(name="apool", bufs=1))
        khs = apool.tile([P, NH, T], BF, name="khs")
        vhs = apool.tile([P, G, NH * HD], BF, name="vhs")   # [keys, kb, 4h*d]
        y_sb = apool.tile([P, NH, T], BF, name="y_sb")
        qj_all = apool.tile([P, NTB, NH, TB], BF, name="qj_all")

        wopool = ctx.enter_context(tc.tile_pool(name="wop", bufs=1))
        wo_dq = wopool.tile([P, NH, D], BF, name="wo_dq")

        # ================= phase A: dequant + QKV + rope ===============
        with ExitStack() as pctx:
            wpool = pctx.enter_context(tc.tile_pool(name="wdqp", bufs=1))
            w_dq = {
                "q": wpool.tile([P, G, OSH], BF, name="wq_dq"),
                "k": wpool.tile([P, G, OSH], BF, name="wk_dq"),
                "v": wpool.tile([P, G, OSH], BF, name="wv_dq"),
            }
            dq = pctx.enter_context(tc.tile_pool(name="dq", bufs=3))
            first = [True]

            def dequant_all(pr):
                for m in range(G // MEGA):
                    gs = [m * MEGA + i for i in range(MEGA)]
                    dequant_mega(
                        dq, [w_views[pr][g] for g in gs],
                        w_dq[pr][:, gs[0]:gs[-1] + 1].rearrange(
                            "p g o -> p (g o)"),
                        MEGA * OSH)
                    if first[0]:
                        first[0] = False
                        nc.sync.dma_start(tri[:], tri_in[:])
                        nc.sync.dma_start(ropeC[:], ropeC_in[:])
                        nc.sync.dma_start(ropeS[:], ropeS_in[:])

            for pr in ("q", "k", "v"):
                dequant_all(pr)
            while dq_pend:
                dequant_back()

            # wo dequant chunks, spread across the tb sections below so the
            # DVE back-half never bunches against rope evac work
            wo_chunks = [(fc, m * MEGA * OSH) for fc in range(NH)
                         for m in range(4 // MEGA)]

            def emit_wo_chunks(n):
                for _ in range(n):
                    if not wo_chunks:
                        break
                    fc, oc0 = wo_chunks.pop(0)
                    dequant_mega(
                        dq,
                        [woT_v[fc, :, oc0 + i * OSH:oc0 + (i + 1) * OSH]
                         for i in range(MEGA)],
                        wo_dq[:, fc, oc0:oc0 + MEGA * OSH], MEGA * OSH)
                if not wo_chunks:
                    while dq_pend:
                        dequant_back()

            xpool = pctx.enter_context(tc.tile_pool(name="xp", bufs=2))
            evac = pctx.enter_context(tc.tile_pool(name="evac", bufs=3))
            psQK = pctx.enter_context(
                tc.tile_pool(name="psQK", bufs=1, space="PSUM"))
            psV = pctx.enter_context(
                tc.tile_pool(name="psV", bufs=2, space="PSUM"))

            for tb in range(NTB):
                ts = slice(tb * TB, (tb + 1) * TB)
                xTr = xpool.tile([P, G, TB], BF, tag="xTr", name="xTr")
                for g in range(G):
                    nc.sync.dma_start(xTr[:, g], xT_v[g, :, ts])

                # q, k projections: g-outer / head-inner so PE consumption
                # paces the dequant stream on tb=0.
                for pr in ("q", "k"):
                    pqs = [psQK.tile([P, TB], F32, tag=f"pq{h}", name=f"pq{h}")
                           for h in range(NH)]
                    for g in range(G):
                        for h in range(NH):
                            nc.tensor.matmul(
                                pqs[h][:], w_dq[pr][:, g, h * HD:(h + 1) * HD],
                                xTr[:, g], start=(g == 0), stop=(g == G - 1))
                    for h in range(NH):
                        qe = evac.tile([P, TB], BF, tag="qe", name="qe")
                        nc.scalar.copy(qe[:], pqs[h][:])
                        qa = evac.tile([P, TB], BF, tag="qa", name="qa")
                        nc.vector.tensor_tensor(qa[:], qe[:],
                                                ropeC[:, ts], ALU.mult)
                        qs = evac.tile([P, TB], BF, tag="qs", name="qs")
                        nc.vector.tensor_tensor(qs[:], qe[:],
                                                ropeS[:, ts], ALU.mult)
                        # rope half-swap: SBUF->SBUF DMA (engines can't cross
                        # partition offsets), then one same-partition add
                        tsw = evac.tile([P, TB], BF, tag="tsw", name="tsw")
                        nc.sync.dma_start(tsw[0:64], qs[64:128])
                        nc.sync.dma_start(tsw[64:128], qs[0:64])
                        if pr == "q":
                            dst = qj_all[:, tb, h]
                        else:
                            dst = khs[:, h, tb * TB:(tb + 1) * TB]
                        nc.vector.tensor_tensor(dst, qa[:], tsw[:], ALU.add)

                # v projection, output directly [keys, kb, vdims] in SBUF
                for tk in range(NTB):
                    pv = psV.tile([P, TB], F32, tag="pv", name="pv")
                    for g in range(G):
                        nc.tensor.matmul(
                            pv[:], xTr[:, g, tk * HD:(tk + 1) * HD],
                            w_dq["v"][:, g], start=(g == 0), stop=(g == G - 1))
                    nc.scalar.copy(vhs[:, tb * NTB + tk], pv[:])

                if tb >= 1:
                    emit_wo_chunks(6)

        # ============ phase B: attention + o_proj ======================
        with ExitStack() as pctx:
            expool = pctx.enter_context(tc.tile_pool(name="exp", bufs=6))
            sspool = pctx.enter_context(tc.tile_pool(name="ss", bufs=4))
            opool = pctx.enter_context(tc.tile_pool(name="op", bufs=4))
            psS = pctx.enter_context(
                tc.tile_pool(name="psS", bufs=4, space="PSUM"))
            psY = pctx.enter_context(
                tc.tile_pool(name="psY", bufs=2, space="PSUM"))
            psD = pctx.enter_context(
                tc.tile_pool(name="psD", bufs=1, space="PSUM"))
            psO = pctx.enter_context(
                tc.tile_pool(name="psO", bufs=1, space="PSUM"))

            def oproj_blocks(tb, obs):
                ts = slice(tb * TB, (tb + 1) * TB)
                for ob in obs:
                    ps_o = psO.tile([P, TB], F32, tag="ps_o", name="ps_o")
                    for fc in range(NH):
                        nc.tensor.matmul(
                            ps_o[:], wo_dq[:, fc, ob * P:(ob + 1) * P],
                            y_sb[:, fc, ts],
                            start=(fc == 0), stop=(fc == NH - 1))
                    ot = opool.tile([P, TB], BF, tag="ot", name="ot")
                    if ob % 2 == 0:
                        nc.scalar.copy(ot[:], ps_o[:])
                    else:
                        nc.vector.tensor_copy(ot[:], ps_o[:])
                    nc.sync.dma_start(outT_v[ob, :, ts], ot[:])

            for j in range(NTB):
                ts = slice(j * TB, (j + 1) * TB)
                nkk = 4 * j + 4
                for h in range(NH):
                    qj = qj_all[:, j, h]
                    ps_y = psY.tile([P, TB], F32, tag="py", name="py")
                    exs = sspool.tile([P, TB], BF, tag="exs", name="exs")
                    pend = []  # (kk, ex, off, ncols)

                    def flush_pv(kk, ex, off, ncols):
                        nc.tensor.matmul(ps_y[:, off:],
                                         vhs[:, kk, h * HD:(h + 1) * HD],
                                         ex[:, 0:ncols],
                                         start=(kk == 0), stop=(kk == nkk - 1))

                    for kk in range(nkk):
                        d = kk - 4 * j
                        off = P * d if d >= 0 else 0
                        ncols = TB - off
                        ps_st = psS.tile([P, TB], F32, tag="st", name="st")
                        st = ps_st[:, 0:ncols]
                        nc.tensor.matmul(
                            st, khs[:, h, kk * P:(kk + 1) * P],
                            qj[:, off:TB], start=True, stop=True)
                        ex = expool.tile([P, TB], BF, tag="ex", name="ex")
                        nc.scalar.activation(ex[:, 0:ncols], st, AF.Exp,
                                             scale=SCALE)
                        if d >= 0:
                            # causal mask as post-exp 0/1 multiply: cheaper
                            # (bf16 SBUF in-place) than the -1e30 PSUM add
                            nc.vector.tensor_tensor(ex[:, 0:P], ex[:, 0:P],
                                                    tri[:], ALU.mult)
                        if kk == 0:
                            nc.vector.tensor_copy(exs[:], ex[:])
                        else:
                            nc.vector.tensor_tensor(
                                exs[:, off:], exs[:, off:], ex[:, 0:ncols],
                                ALU.add)
                        pend.append((kk, ex, off, ncols))
                        if len(pend) > 3:
                            flush_pv(*pend.pop(0))
                    while pend:
                        flush_pv(*pend.pop(0))

                    ps_den = psD.tile([1, TB], F32, tag="pd", name="pd")
                    nc.tensor.matmul(ps_den[:], ones_b[:], exs[:],
                                     start=True, stop=True)
                    rec = expool.tile([1, TB], BF, tag="rec", name="rec")
                    with nc.allow_low_precision("bf16 1/denom"):
                        nc.vector.reciprocal(rec[:], ps_den[:])
                    den_b = sspool.tile([P, TB], BF, tag="den_b",
                                        name="den_b")
                    nc.gpsimd.partition_broadcast(den_b[:], rec[:])
                    nc.vector.tensor_tensor(y_sb[:, h, ts], ps_y[:],
                                            den_b[:], ALU.mult)
                oproj_blocks(j, range(G))

    from concourse import library_overlay
    library_overlay.lower_extended_insts(nc)
    return nc


def _rope_tables():
    half = HD // 2
    inv_freq = 1.0 / (10000.0 ** (np.arange(half, dtype=np.float64) / half))
    freqs = np.outer(np.arange(T, dtype=np.float64), inv_freq)  # [T, 64]
    c = np.cos(freqs).T  # [64, T]
    s = np.sin(freqs).T
    # S'' = [sin; -sin]: rope = q*C + swap_halves(q*S'')
    return (np.concatenate([c, c], axis=0),
            np.concatenate([s, -s], axis=0))


def kernel(x, w_q, w_k, w_v, w_o):
    _install_waitfix()
    from concourse.bass_utils import run_bass_kernel_spmd
    import ml_dtypes

    bf16 = ml_dtypes.bfloat16
    x = np.asarray(x, dtype=np.float32)
    w_q = np.asarray(w_q, dtype=np.float32)
    w_k = np.asarray(w_k, dtype=np.float32)
    w_v = np.asarray(w_v, dtype=np.float32)
    w_o = np.asarray(w_o, dtype=np.float32)
    B = x.shape[0]

    if "nc" not in _cached:
        _cached["nc"] = _build_nc()
    nc = _cached["nc"]

    ropeC, ropeS = _rope_tables()
    idx = np.arange(P)
    tri = np.where(idx[:, None] > idx[None, :], np.float32(0.0),
                   np.float32(1.0))

    in_maps = []
    for c in range(8):
        b, hg = divmod(c, 4)
        osl = slice(hg * OSH, (hg + 1) * OSH)
        in_maps.append({
            "xT": np.ascontiguousarray(x[b].T).astype(bf16),
            "wqT": np.ascontiguousarray(w_q[osl, :].T),
            "wkT": np.ascontiguousarray(w_k[osl, :].T),
            "wvT": np.ascontiguousarray(w_v[osl, :].T),
            "woT": np.ascontiguousarray(w_o[:, osl].T),
            "ropeC": ropeC.astype(bf16), "ropeS": ropeS.astype(bf16),
            "tri": tri.astype(bf16),
        })

    import os as _os
    trace = _os.environ.get("BITATTN_TRACE") == "1"
    res = run_bass_kernel_spmd(nc, in_maps, core_ids=list(range(8)),
                               trace=trace)
    _cached["last_res"] = res
    out = np.zeros((B, T, D), dtype=np.float32)
    for c in range(8):
        b = c // 4
        out[b] += res.results[c]["outT"].astype(np.float32).T
    return out


# revision 11
# speedup vs baseline: 64.6840x; 1.0015x over previous
"""BitSelfAttention TRN2 kernel v2 (8 NeuronCores, TP over heads + batch over B).

Core c -> batch b=c//4, head group hg=c%4 (heads 4*hg..4*hg+3).
All matmuls bf16 (1 cy/row); dequant threshold math kept f32-exact:
scale sum via gpsimd partition_all_reduce (f32), compare via DVE STT.
Rope half-swap baked into the DRAM store (swapped row ranges); reload adds
qa + swap(qs) on DVE in bf16.  Softmax denominator via DVE-accumulated exp
sums + one [1,512] matmul per (j,h).  o_proj PSUM evac on Pool engine.
Self-contained; includes the walrus one-wait BIR legalizer.
"""
import json
import numpy as np

# ---------------------------------------------------------------- constants
P = 128
T = 2048
D = 2048
NH = 4                     # heads per core
HD = 128                   # head dim
TB = 512                   # t-block
NTB = T // TB              # 4
G = D // P                 # 16 contraction chunks
OSH = 512                  # per-core qkv output-column shard
SCALE = HD ** -0.5
NEG = -1e30
MEGA = 1                   # dequant chunks merged per op

_cached = {}


# ------------------------------------------------------------- BIR legalizer
def _legalize_bir_json(bir_json: bytes) -> bytes:
    """This walrus accepts only ONE sync-wait (and update) per instruction.
    Hoist extras onto same-engine NoOps (engine FIFO keeps semantics)."""
    m = json.loads(bir_json)
    n = [0]

    def nop(engine, waits, updates):
        n[0] += 1
        return {"name": f"I-wfix{n[0]}", "opcode": "NoOp", "engine": engine,
                "ins": [], "outs": [],
                "sync_info": {"on_wait": waits, "on_update": updates}}

    for f in m.get("functions", []):
        for blk in f.get("blocks", []):
            out = []
            for inst in blk.get("instructions", []):
                si = inst.get("sync_info")
                if not si:
                    out.append(inst)
                    continue
                waits = si.get("on_wait") or []
                ups = si.get("on_update") or []
                post = []
                if len(waits) > 1:
                    for w in waits[:-1]:
                        out.append(nop(inst["engine"], [w], []))
                    si["on_wait"] = [waits[-1]]
                if len(ups) > 1:
                    assert inst.get("opcode") not in (
                        "DMACopy", "DMATranspose", "DMAGather",
                        "DMAScatterAdd", "TriggerDma"), inst.get("name")
                    si["on_update"] = [ups[0]]
                    for u in ups[1:]:
                        post.append(nop(inst["engine"], [], [u]))
                out.append(inst)
                out.extend(post)
            blk["instructions"] = out
    return json.dumps(m).encode()


def _install_waitfix():
    import concourse.bass_utils as bu
    if getattr(bu, "_bitattn_waitfix", False):
        return
    bu._bitattn_waitfix = True
    orig = bu.compile_bir_kernel

    def patched(bir_json, tmpdir, neff_name="file.neff"):
        return orig(_legalize_bir_json(bir_json), tmpdir, neff_name)

    bu.compile_bir_kernel = patched
    try:
        import concourse.bass2jax as b2j
        if getattr(b2j, "compile_bir_kernel", None) is orig:
            b2j.compile_bir_kernel = patched
    except ImportError:
        pass


# ---------------------------------------------------------------- bass build
def _build_nc():
    import concourse.bass as bass
    import concourse.mybir as mybir
    import concourse.tile as tile
    import concourse.bass_isa as bass_isa
    from concourse import library_config
    from contextlib import ExitStack

    F32 = mybir.dt.float32
    BF = mybir.dt.bfloat16
    AF = mybir.ActivationFunctionType
    ALU = mybir.AluOpType
    RED = bass_isa.ReduceOp

    nc = bass.Bass(name="bitattn2", trn_type="TRN2")
    xT_in = nc.dram_tensor("xT", [D, T], BF, kind="ExternalInput")
    wqT_in = nc.dram_tensor("wqT", [D, OSH], F32, kind="ExternalInput")
    wkT_in = nc.dram_tensor("wkT", [D, OSH], F32, kind="ExternalInput")
    wvT_in = nc.dram_tensor("wvT", [D, OSH], F32, kind="ExternalInput")
    woT_in = nc.dram_tensor("woT", [OSH, D], F32, kind="ExternalInput")
    ropeC_in = nc.dram_tensor("ropeC", [P, T], BF, kind="ExternalInput")
    ropeS_in = nc.dram_tensor("ropeS", [P, T], BF, kind="ExternalInput")
    tri_in = nc.dram_tensor("tri", [P, P], BF, kind="ExternalInput")
    outT = nc.dram_tensor("outT", [D, T], BF, kind="ExternalOutput")

    xT_v = xT_in[:].rearrange("(g p) t -> g p t", p=P)
    w_views = {
        "q": wqT_in[:].rearrange("(g p) o -> g p o", p=P),
        "k": wkT_in[:].rearrange("(g p) o -> g p o", p=P),
        "v": wvT_in[:].rearrange("(g p) o -> g p o", p=P),
    }
    woT_v = woT_in[:].rearrange("(fc p) o -> fc p o", p=P)
    outT_v = outT[:].rearrange("(ob p) t -> ob p t", p=P)

    with tile.TileContext(nc) as tc, ExitStack() as ctx:
        nc.gpsimd.load_library(library_config.attn)

        glob = ctx.enter_context(tc.tile_pool(name="glob", bufs=1))
        ones_b = glob.tile([P, 1], BF)
        nc.gpsimd.memset(ones_b[:], 1.0)
        tri = glob.tile([P, P], BF)
        ropeC = glob.tile([P, T], BF)
        ropeS = glob.tile([P, T], BF)

        # ---------------- dequant mega-chunk helper --------------------
        # For MEGA groups side by side: [128, MEGA*OSH].  Per column o of
        # group g: s = sum_p(2|w|)/256; keep if 2|w| > s; W_dq = sign(w)*s.
        # partition_all_reduce keeps per-column sums independent, so two
        # groups can share one op along the free dim.
        dq_pend = []

        def dequant_front(dq, src_aps, dst_ap, fw):
            wt = dq.tile([P, fw], F32, tag="wt", name="wt")
            step = fw // len(src_aps)
            for i, ap in enumerate(src_aps):
                nc.sync.dma_start(wt[:, i * step:(i + 1) * step], ap)
            # abs scaled 1/128 so the f32 all-reduce is exactly s=mean|w|;
            # compare flips to ab2*256 > sbr (exact powers of 2).  Second
            # all-reduce in bf16 feeds the smooth scale multiplies in 2x.
            ab2 = dq.tile([P, fw], F32, tag="ab2", name="ab2")
            nc.scalar.activation(ab2[:], wt[:], AF.Abs, scale=1.0 / 128.0)
            sbr = dq.tile([P, fw], F32, tag="sbr", name="sbr")
            nc.gpsimd.partition_all_reduce(sbr[:], ab2[:], channels=P,
                                           reduce_op=RED.add)
            sbb = dq.tile([P, fw], BF, tag="sbb", name="sbb")
            nc.gpsimd.partition_all_reduce(sbb[:], ab2[:], channels=P,
                                           reduce_op=RED.add)
            m01 = dq.tile([P, fw], BF, tag="m01", name="m01")
            nc.vector.scalar_tensor_tensor(m01[:], ab2[:], 256.0,
                                           sbr[:], ALU.mult, ALU.is_gt)
            sgn = dq.tile([P, fw], BF, tag="sgn", name="sgn")
            nc.scalar.activation(sgn[:], wt[:], AF.Sign)
            dq_pend.append((dq, sbb, m01, sgn, dst_ap, fw))

        dq_n = [0]

        def dequant_back():
            dq, sbb, m01, sgn, dst_ap, fw = dq_pend.pop(0)
            ms = dq.tile([P, fw], BF, tag="m01", name="ms")
            nc.vector.tensor_tensor(ms[:], sbb[:], m01[:], ALU.mult)
            nc.vector.tensor_tensor(dst_ap, ms[:], sgn[:], ALU.mult)

        # emit back-half one chunk late: DVE never queues a mul that waits
        # on the same chunk's Pool ms (head-of-line stall)
        def dequant_mega(dq, src_aps, dst_ap, fw):
            dequant_front(dq, src_aps, dst_ap, fw)
            if len(dq_pend) > 1:
                dequant_back()

        # attention operands built in SBUF during phase A — no DRAM round trip
        apool = ctx.enter_context(tc.tile_pool(name="apool", bufs=1))
        khs = apool.tile([P, NH, T], BF, name="khs")
        vhs = apool.tile([P, G, NH * HD], BF, name="vhs")   # [keys, kb, 4h*d]
        y_sb = apool.tile([P, NH, T], BF, name="y_sb")
        qj_all = apool.tile([P, NTB, NH, TB], BF, name="qj_all")

        wopool = ctx.enter_context(tc.tile_pool(name="wop", bufs=1))
        wo_dq = wopool.tile([P, NH, D], BF, name="wo_dq")

        # ================= phase A: dequant + QKV + rope ===============
        with ExitStack() as pctx:
            wpool = pctx.enter_context(tc.tile_pool(name="wdqp", bufs=1))
            w_dq = {
                "q": wpool.tile([P, G, OSH], BF, name="wq_dq"),
                "k": wpool.tile([P, G, OSH], BF, name="wk_dq"),
                "v": wpool.tile([P, G, OSH], BF, name="wv_dq"),
            }
            dq = pctx.enter_context(tc.tile_pool(name="dq", bufs=3))
            first = [True]

            def dequant_all(pr):
                for m in range(G // MEGA):
                    gs = [m * MEGA + i for i in range(MEGA)]
                    dequant_mega(
                        dq, [w_views[pr][g] for g in gs],
                        w_dq[pr][:, gs[0]:gs[-1] + 1].rearrange(
                            "p g o -> p (g o)"),
                        MEGA * OSH)
                    if first[0]:
                        first[0] = False
                        nc.sync.dma_start(tri[:], tri_in[:])
                        nc.sync.dma_start(ropeC[:], ropeC_in[:])
                        nc.sync.dma_start(ropeS[:], ropeS_in[:])

            for pr in ("q", "k", "v"):
                dequant_all(pr)
            while dq_pend:
                dequant_back()

            # wo dequant chunks, spread across the tb sections below so the
            # DVE back-half never bunches against rope evac work
            wo_chunks = [(fc, m * MEGA * OSH) for fc in range(NH)
                         for m in range(4 // MEGA)]

            def emit_wo_chunks(n):
                for _ in range(n):
                    if not wo_chunks:
                        break
                    fc, oc0 = wo_chunks.pop(0)
                    dequant_mega(
                        dq,
                        [woT_v[fc, :, oc0 + i * OSH:oc0 + (i + 1) * OSH]
                         for i in range(MEGA)],
                        wo_dq[:, fc, oc0:oc0 + MEGA * OSH], MEGA * OSH)
                if not wo_chunks:
                    while dq_pend:
                        dequant_back()

            xpool = pctx.enter_context(tc.tile_pool(name="xp", bufs=2))
            evac = pctx.enter_context(tc.tile_pool(name="evac", bufs=3))
            psQK = pctx.enter_context(
                tc.tile_pool(name="psQK", bufs=1, space="PSUM"))
            psV = pctx.enter_context(
                tc.tile_pool(name="psV", bufs=2, space="PSUM"))

            for tb in range(NTB):
                ts = slice(tb * TB, (tb + 1) * TB)
                xTr = xpool.tile([P, G, TB], BF, tag="xTr", name="xTr")
                for g in range(G):
                    nc.sync.dma_start(xTr[:, g], xT_v[g, :, ts])

                # q, k projections: g-outer / head-inner so PE consumption
                # paces the dequant stream on tb=0.
                for pr in ("q", "k"):
                    pqs = [psQK.tile([P, TB], F32, tag=f"pq{h}", name=f"pq{h}")
                           for h in range(NH)]
                    for g in range(G):
                        for h in range(NH):
                            nc.tensor.matmul(
                                pqs[h][:], w_dq[pr][:, g, h * HD:(h + 1) * HD],
                                xTr[:, g], start=(g == 0), stop=(g == G - 1))
                    for h in range(NH):
                        qe = evac.tile([P, TB], BF, tag="qe", name="qe")
                        nc.scalar.copy(qe[:], pqs[h][:])
                        qa = evac.tile([P, TB], BF, tag="qa", name="qa")
                        nc.vector.tensor_tensor(qa[:], qe[:],
                                                ropeC[:, ts], ALU.mult)
                        qs = evac.tile([P, TB], BF, tag="qs", name="qs")
                        nc.vector.tensor_tensor(qs[:], qe[:],
                                                ropeS[:, ts], ALU.mult)
                        # rope half-swap: SBUF->SBUF DMA (engines can't cross
                        # partition offsets), then one same-partition add
                        tsw = evac.tile([P, TB], BF, tag="tsw", name="tsw")
                        nc.sync.dma_start(tsw[0:64], qs[64:128])
                        nc.sync.dma_start(tsw[64:128], qs[0:64])
                        if pr == "q":
                            dst = qj_all[:, tb, h]
                        else:
                            dst = khs[:, h, tb * TB:(tb + 1) * TB]
                        nc.vector.tensor_tensor(dst, qa[:], tsw[:], ALU.add)

                # v projection, output directly [keys, kb, vdims] in SBUF
                for tk in range(NTB):
                    pv = psV.tile([P, TB], F32, tag="pv", name="pv")
                    for g in range(G):
                        nc.tensor.matmul(
                            pv[:], xTr[:, g, tk * HD:(tk + 1) * HD],
                            w_dq["v"][:, g], start=(g == 0), stop=(g == G - 1))
                    nc.scalar.copy(vhs[:, tb * NTB + tk], pv[:])

                if tb >= 1:
                    emit_wo_chunks(6)

        # ============ phase B: attention + o_proj ======================
        with ExitStack() as pctx:
            expool = pctx.enter_context(tc.tile_pool(name="exp", bufs=8))
            sspool = pctx.enter_context(tc.tile_pool(name="ss", bufs=4))
            opool = pctx.enter_context(tc.tile_pool(name="op", bufs=4))
            psS = pctx.enter_context(
                tc.tile_pool(name="psS", bufs=4, space="PSUM"))
            psY = pctx.enter_context(
                tc.tile_pool(name="psY", bufs=2, space="PSUM"))
            psD = pctx.enter_context(
                tc.tile_pool(name="psD", bufs=1, space="PSUM"))
            psO = pctx.enter_context(
                tc.tile_pool(name="psO", bufs=1, space="PSUM"))

            def oproj_blocks(tb, obs):
                ts = slice(tb * TB, (tb + 1) * TB)
                for ob in obs:
                    ps_o = psO.tile([P, TB], F32, tag="ps_o", name="ps_o")
                    for fc in range(NH):
                        nc.tensor.matmul(
                            ps_o[:], wo_dq[:, fc, ob * P:(ob + 1) * P],
                            y_sb[:, fc, ts],
                            start=(fc == 0), stop=(fc == NH - 1))
                    ot = opool.tile([P, TB], BF, tag="ot", name="ot")
                    if ob % 2 == 0:
                        nc.scalar.copy(ot[:], ps_o[:])
                    else:
                        nc.vector.tensor_copy(ot[:], ps_o[:])
                    nc.sync.dma_start(outT_v[ob, :, ts], ot[:])

            for j in range(NTB):
                ts = slice(j * TB, (j + 1) * TB)
                nkk = 4 * j + 4
                for h in range(NH):
                    qj = qj_all[:, j, h]
                    ps_y = psY.tile([P, TB], F32, tag="py", name="py")
                    exs = sspool.tile([P, TB], BF, tag="exs", name="exs")
                    pend = []  # (kk, ex, off, ncols)

                    def flush_pv(kk, ex, off, ncols):
                        nc.tensor.matmul(ps_y[:, off:],
                                         vhs[:, kk, h * HD:(h + 1) * HD],
                                         ex[:, 0:ncols],
                                         start=(kk == 0), stop=(kk == nkk - 1))

                    for kk in range(nkk):
                        d = kk - 4 * j
                        off = P * d if d >= 0 else 0
                        ncols = TB - off
                        ps_st = psS.tile([P, TB], F32, tag="st", name="st")
                        st = ps_st[:, 0:ncols]
                        nc.tensor.matmul(
                            st, khs[:, h, kk * P:(kk + 1) * P],
                            qj[:, off:TB], start=True, stop=True)
                        ex = expool.tile([P, TB], BF, tag="ex", name="ex")
                        nc.scalar.activation(ex[:, 0:ncols], st, AF.Exp,
                                             scale=SCALE)
                        if d >= 0:
                            # causal mask as post-exp 0/1 multiply: cheaper
                            # (bf16 SBUF in-place) than the -1e30 PSUM add
                            nc.vector.tensor_tensor(ex[:, 0:P], ex[:, 0:P],
                                                    tri[:], ALU.mult)
                        if kk == 0:
                            nc.vector.tensor_copy(exs[:], ex[:])
                        else:
                            nc.vector.tensor_tensor(
                                exs[:, off:], exs[:, off:], ex[:, 0:ncols],
                                ALU.add)
                        pend.append((kk, ex, off, ncols))
                        if len(pend) > 3:
                            flush_pv(*pend.pop(0))
                    while pend:
                        flush_pv(*pend.pop(0))

                    ps_den = psD.tile([1, TB], F32, tag="pd", name="pd")
                    nc.tensor.matmul(ps_den[:], ones_b[:], exs[:],
                                     start=True, stop=True)
                    rec = expool.tile([1, TB], BF, tag="rec", name="rec")
                    with nc.allow_low_precision("bf16 1/denom"):
                        nc.vector.reciprocal(rec[:], ps_den[:])
                    den_b = sspool.tile([P, TB], BF, tag="den_b",
                                        name="den_b")
                    nc.gpsimd.partition_broadcast(den_b[:], rec[:])
                    nc.vector.tensor_tensor(y_sb[:, h, ts], ps_y[:],
                                            den_b[:], ALU.mult)
                oproj_blocks(j, range(G))

    from concourse import library_overlay
    library_overlay.lower_extended_insts(nc)
    return nc


def _rope_tables():
    half = HD // 2
    inv_freq = 1.0 / (10000.0 ** (np.arange(half, dtype=np.float64) / half))
    freqs = np.outer(np.arange(T, dtype=np.float64), inv_freq)  # [T, 64]
    c = np.cos(freqs).T  # [64, T]
    s = np.sin(freqs).T
    # S'' = [sin; -sin]: rope = q*C + swap_halves(q*S'')
    return (np.concatenate([c, c], axis=0),
            np.concatenate([s, -s], axis=0))


def kernel(x, w_q, w_k, w_v, w_o):
    _install_waitfix()
    from concourse.bass_utils import run_bass_kernel_spmd
    import ml_dtypes

    bf16 = ml_dtypes.bfloat16
    x = np.asarray(x, dtype=np.float32)
    w_q = np.asarray(w_q, dtype=np.float32)
    w_k = np.asarray(w_k, dtype=np.float32)
    w_v = np.asarray(w_v, dtype=np.float32)
    w_o = np.asarray(w_o, dtype=np.float32)
    B = x.shape[0]

    if "nc" not in _cached:
        _cached["nc"] = _build_nc()
    nc = _cached["nc"]

    ropeC, ropeS = _rope_tables()
    idx = np.arange(P)
    tri = np.where(idx[:, None] > idx[None, :], np.float32(0.0),
                   np.float32(1.0))

    in_maps = []
    for c in range(8):
        b, hg = divmod(c, 4)
        osl = slice(hg * OSH, (hg + 1) * OSH)
        in_maps.append({
            "xT": np.ascontiguousarray(x[b].T).astype(bf16),
            "wqT": np.ascontiguousarray(w_q[osl, :].T),
            "wkT": np.ascontiguousarray(w_k[osl, :].T),
            "wvT": np.ascontiguousarray(w_v[osl, :].T),
            "woT": np.ascontiguousarray(w_o[:, osl].T),
            "ropeC": ropeC.astype(bf16), "ropeS": ropeS.astype(bf16),
            "tri": tri.astype(bf16),
        })

    import os as _os
    trace = _os.environ.get("BITATTN_TRACE") == "1"
    res = run_bass_kernel_spmd(nc, in_maps, core_ids=list(range(8)),
                               trace=trace)
    _cached["last_res"] = res
    out = np.zeros((B, T, D), dtype=np.float32)
    for c in range(8):
        b = c // 4
        out[b] += res.results[c]["outT"].astype(np.float32).T
    return out


# revision 12
# speedup vs baseline: 67.6348x; 1.0456x over previous
"""BitSelfAttention TRN2 kernel v2 (8 NeuronCores, TP over heads + batch over B).

Core c -> batch b=c//4, head group hg=c%4 (heads 4*hg..4*hg+3).
All matmuls bf16 (1 cy/row); dequant threshold math kept f32-exact:
scale sum via gpsimd partition_all_reduce (f32), compare via DVE STT.
Rope half-swap baked into the DRAM store (swapped row ranges); reload adds
qa + swap(qs) on DVE in bf16.  Softmax denominator via DVE-accumulated exp
sums + one [1,512] matmul per (j,h).  o_proj PSUM evac on Pool engine.
Self-contained; includes the walrus one-wait BIR legalizer.
"""
import json
import numpy as np

# ---------------------------------------------------------------- constants
P = 128
T = 2048
D = 2048
NH = 4                     # heads per core
HD = 128                   # head dim
TB = 512                   # t-block
NTB = T // TB              # 4
G = D // P                 # 16 contraction chunks
OSH = 512                  # per-core qkv output-column shard
SCALE = HD ** -0.5
NEG = -1e30
MEGA = 1                   # dequant chunks merged per op

_cached = {}


# ------------------------------------------------------------- BIR legalizer
def _legalize_bir_json(bir_json: bytes) -> bytes:
    """This walrus accepts only ONE sync-wait (and update) per instruction.
    Hoist extras onto same-engine NoOps (engine FIFO keeps semantics)."""
    m = json.loads(bir_json)
    n = [0]

    def nop(engine, waits, updates):
        n[0] += 1
        return {"name": f"I-wfix{n[0]}", "opcode": "NoOp", "engine": engine,
                "ins": [], "outs": [],
                "sync_info": {"on_wait": waits, "on_update": updates}}

    for f in m.get("functions", []):
        for blk in f.get("blocks", []):
            out = []
            for inst in blk.get("instructions", []):
                si = inst.get("sync_info")
                if not si:
                    out.append(inst)
                    continue
                waits = si.get("on_wait") or []
                ups = si.get("on_update") or []
                post = []
                if len(waits) > 1:
                    for w in waits[:-1]:
                        out.append(nop(inst["engine"], [w], []))
                    si["on_wait"] = [waits[-1]]
                if len(ups) > 1:
                    assert inst.get("opcode") not in (
                        "DMACopy", "DMATranspose", "DMAGather",
                        "DMAScatterAdd", "TriggerDma"), inst.get("name")
                    si["on_update"] = [ups[0]]
                    for u in ups[1:]:
                        post.append(nop(inst["engine"], [], [u]))
                out.append(inst)
                out.extend(post)
            blk["instructions"] = out
    return json.dumps(m).encode()


def _install_waitfix():
    import concourse.bass_utils as bu
    if getattr(bu, "_bitattn_waitfix", False):
        return
    bu._bitattn_waitfix = True
    orig = bu.compile_bir_kernel

    def patched(bir_json, tmpdir, neff_name="file.neff"):
        return orig(_legalize_bir_json(bir_json), tmpdir, neff_name)

    bu.compile_bir_kernel = patched
    try:
        import concourse.bass2jax as b2j
        if getattr(b2j, "compile_bir_kernel", None) is orig:
            b2j.compile_bir_kernel = patched
    except ImportError:
        pass


# ---------------------------------------------------------------- bass build
def _build_nc():
    import concourse.bass as bass
    import concourse.mybir as mybir
    import concourse.tile as tile
    import concourse.bass_isa as bass_isa
    from concourse import library_config
    from contextlib import ExitStack

    F32 = mybir.dt.float32
    BF = mybir.dt.bfloat16
    AF = mybir.ActivationFunctionType
    ALU = mybir.AluOpType
    RED = bass_isa.ReduceOp

    nc = bass.Bass(name="bitattn2", trn_type="TRN2")
    xT_in = nc.dram_tensor("xT", [D, T], BF, kind="ExternalInput")
    wqT_in = nc.dram_tensor("wqT", [D, OSH], F32, kind="ExternalInput")
    wkT_in = nc.dram_tensor("wkT", [D, OSH], F32, kind="ExternalInput")
    wvT_in = nc.dram_tensor("wvT", [D, OSH], F32, kind="ExternalInput")
    woT_in = nc.dram_tensor("woT", [OSH, D], F32, kind="ExternalInput")
    ropeC_in = nc.dram_tensor("ropeC", [P, T], BF, kind="ExternalInput")
    ropeS_in = nc.dram_tensor("ropeS", [P, T], BF, kind="ExternalInput")
    tri_in = nc.dram_tensor("tri", [P, P], BF, kind="ExternalInput")
    outT = nc.dram_tensor("outT", [D, T], BF, kind="ExternalOutput")

    xT_v = xT_in[:].rearrange("(g p) t -> g p t", p=P)
    w_views = {
        "q": wqT_in[:].rearrange("(g p) o -> g p o", p=P),
        "k": wkT_in[:].rearrange("(g p) o -> g p o", p=P),
        "v": wvT_in[:].rearrange("(g p) o -> g p o", p=P),
    }
    woT_v = woT_in[:].rearrange("(fc p) o -> fc p o", p=P)
    outT_v = outT[:].rearrange("(ob p) t -> ob p t", p=P)

    with tile.TileContext(nc) as tc, ExitStack() as ctx:
        nc.gpsimd.load_library(library_config.attn)

        glob = ctx.enter_context(tc.tile_pool(name="glob", bufs=1))
        ones_b = glob.tile([P, 1], BF)
        nc.gpsimd.memset(ones_b[:], 1.0)
        tri = glob.tile([P, P], BF)
        ropeC = glob.tile([P, T], BF)
        ropeS = glob.tile([P, T], BF)

        # ---------------- dequant mega-chunk helper --------------------
        # For MEGA groups side by side: [128, MEGA*OSH].  Per column o of
        # group g: s = sum_p(2|w|)/256; keep if 2|w| > s; W_dq = sign(w)*s.
        # partition_all_reduce keeps per-column sums independent, so two
        # groups can share one op along the free dim.
        dq_pend = []

        def dequant_front(dq, src_aps, dst_ap, fw):
            wt = dq.tile([P, fw], F32, tag="wt", name="wt")
            step = fw // len(src_aps)
            for i, ap in enumerate(src_aps):
                nc.sync.dma_start(wt[:, i * step:(i + 1) * step], ap)
            # abs scaled 1/128 so the f32 all-reduce is exactly s=mean|w|;
            # compare flips to ab2*256 > sbr (exact powers of 2).  Second
            # all-reduce in bf16 feeds the smooth scale multiplies in 2x.
            ab2 = dq.tile([P, fw], F32, tag="ab2", name="ab2")
            nc.scalar.activation(ab2[:], wt[:], AF.Abs, scale=1.0 / 128.0)
            sbr = dq.tile([P, fw], F32, tag="sbr", name="sbr")
            nc.gpsimd.partition_all_reduce(sbr[:], ab2[:], channels=P,
                                           reduce_op=RED.add)
            sbb = dq.tile([P, fw], BF, tag="sbb", name="sbb")
            nc.gpsimd.partition_all_reduce(sbb[:], ab2[:], channels=P,
                                           reduce_op=RED.add)
            m01 = dq.tile([P, fw], BF, tag="m01", name="m01")
            nc.vector.scalar_tensor_tensor(m01[:], ab2[:], 256.0,
                                           sbr[:], ALU.mult, ALU.is_gt)
            sgn = dq.tile([P, fw], BF, tag="sgn", name="sgn")
            nc.scalar.activation(sgn[:], wt[:], AF.Sign)
            dq_pend.append((dq, sbb, m01, sgn, dst_ap, fw))

        dq_n = [0]

        def dequant_back():
            dq, sbb, m01, sgn, dst_ap, fw = dq_pend.pop(0)
            ms = dq.tile([P, fw], BF, tag="m01", name="ms")
            nc.vector.tensor_tensor(ms[:], sbb[:], m01[:], ALU.mult)
            nc.vector.tensor_tensor(dst_ap, ms[:], sgn[:], ALU.mult)

        # emit back-half one chunk late: DVE never queues a mul that waits
        # on the same chunk's Pool ms (head-of-line stall)
        def dequant_mega(dq, src_aps, dst_ap, fw):
            dequant_front(dq, src_aps, dst_ap, fw)
            if len(dq_pend) > 1:
                dequant_back()

        # attention operands built in SBUF during phase A — no DRAM round trip
        apool = ctx.enter_context(tc.tile_pool(name="apool", bufs=1))
        khs = apool.tile([P, NH, T], BF, name="khs")
        vhs = apool.tile([P, G, NH * HD], BF, name="vhs")   # [keys, kb, 4h*d]
        y_sb = apool.tile([P, NH, T], BF, name="y_sb")
        qj_all = apool.tile([P, NTB, NH, TB], BF, name="qj_all")

        wopool = ctx.enter_context(tc.tile_pool(name="wop", bufs=1))
        wo_dq = wopool.tile([P, NH, D], BF, name="wo_dq")

        # ================= phase A: dequant + QKV + rope ===============
        with ExitStack() as pctx:
            wpool = pctx.enter_context(tc.tile_pool(name="wdqp", bufs=1))
            w_dq = {
                "q": wpool.tile([P, G, OSH], BF, name="wq_dq"),
                "k": wpool.tile([P, G, OSH], BF, name="wk_dq"),
                "v": wpool.tile([P, G, OSH], BF, name="wv_dq"),
            }
            dq = pctx.enter_context(tc.tile_pool(name="dq", bufs=3))
            first = [0]

            def dequant_all(pr):
                for m in range(G // MEGA):
                    gs = [m * MEGA + i for i in range(MEGA)]
                    dequant_mega(
                        dq, [w_views[pr][g] for g in gs],
                        w_dq[pr][:, gs[0]:gs[-1] + 1].rearrange(
                            "p g o -> p (g o)"),
                        MEGA * OSH)
                    first[0] += 1
                    if first[0] == 6:
                        nc.sync.dma_start(tri[:], tri_in[:])
                        nc.sync.dma_start(ropeC[:], ropeC_in[:])
                        nc.sync.dma_start(ropeS[:], ropeS_in[:])

            for pr in ("q", "k", "v"):
                dequant_all(pr)
            while dq_pend:
                dequant_back()

            # wo dequant chunks, spread across the tb sections below so the
            # DVE back-half never bunches against rope evac work
            wo_chunks = [(fc, m * MEGA * OSH) for fc in range(NH)
                         for m in range(4 // MEGA)]

            def emit_wo_chunks(n):
                for _ in range(n):
                    if not wo_chunks:
                        break
                    fc, oc0 = wo_chunks.pop(0)
                    dequant_mega(
                        dq,
                        [woT_v[fc, :, oc0 + i * OSH:oc0 + (i + 1) * OSH]
                         for i in range(MEGA)],
                        wo_dq[:, fc, oc0:oc0 + MEGA * OSH], MEGA * OSH)
                if not wo_chunks:
                    while dq_pend:
                        dequant_back()

            xpool = pctx.enter_context(tc.tile_pool(name="xp", bufs=2))
            evac = pctx.enter_context(tc.tile_pool(name="evac", bufs=3))
            psQK = pctx.enter_context(
                tc.tile_pool(name="psQK", bufs=1, space="PSUM"))
            psV = pctx.enter_context(
                tc.tile_pool(name="psV", bufs=2, space="PSUM"))

            for tb in range(NTB):
                ts = slice(tb * TB, (tb + 1) * TB)
                xTr = xpool.tile([P, G, TB], BF, tag="xTr", name="xTr")
                for g in range(G):
                    nc.sync.dma_start(xTr[:, g], xT_v[g, :, ts])

                # q, k projections: g-outer / head-inner so PE consumption
                # paces the dequant stream on tb=0.
                for pr in ("q", "k"):
                    pqs = [psQK.tile([P, TB], F32, tag=f"pq{h}", name=f"pq{h}")
                           for h in range(NH)]
                    for g in range(G):
                        for h in range(NH):
                            nc.tensor.matmul(
                                pqs[h][:], w_dq[pr][:, g, h * HD:(h + 1) * HD],
                                xTr[:, g], start=(g == 0), stop=(g == G - 1))
                    for h in range(NH):
                        qe = evac.tile([P, TB], BF, tag="qe", name="qe")
                        nc.scalar.copy(qe[:], pqs[h][:])
                        qa = evac.tile([P, TB], BF, tag="qa", name="qa")
                        nc.vector.tensor_tensor(qa[:], qe[:],
                                                ropeC[:, ts], ALU.mult)
                        qs = evac.tile([P, TB], BF, tag="qs", name="qs")
                        nc.vector.tensor_tensor(qs[:], qe[:],
                                                ropeS[:, ts], ALU.mult)
                        # rope half-swap: SBUF->SBUF DMA (engines can't cross
                        # partition offsets), then one same-partition add
                        tsw = evac.tile([P, TB], BF, tag="tsw", name="tsw")
                        nc.sync.dma_start(tsw[0:64], qs[64:128])
                        nc.sync.dma_start(tsw[64:128], qs[0:64])
                        if pr == "q":
                            dst = qj_all[:, tb, h]
                        else:
                            dst = khs[:, h, tb * TB:(tb + 1) * TB]
                        nc.vector.tensor_tensor(dst, qa[:], tsw[:], ALU.add)

                # v projection, output directly [keys, kb, vdims] in SBUF
                for tk in range(NTB):
                    pv = psV.tile([P, TB], F32, tag="pv", name="pv")
                    for g in range(G):
                        nc.tensor.matmul(
                            pv[:], xTr[:, g, tk * HD:(tk + 1) * HD],
                            w_dq["v"][:, g], start=(g == 0), stop=(g == G - 1))
                    nc.scalar.copy(vhs[:, tb * NTB + tk], pv[:])

                if tb >= 1:
                    emit_wo_chunks(6)

        # ============ phase B: attention + o_proj ======================
        with ExitStack() as pctx:
            expool = pctx.enter_context(tc.tile_pool(name="exp", bufs=8))
            sspool = pctx.enter_context(tc.tile_pool(name="ss", bufs=4))
            opool = pctx.enter_context(tc.tile_pool(name="op", bufs=4))
            psS = pctx.enter_context(
                tc.tile_pool(name="psS", bufs=4, space="PSUM"))
            psY = pctx.enter_context(
                tc.tile_pool(name="psY", bufs=2, space="PSUM"))
            psD = pctx.enter_context(
                tc.tile_pool(name="psD", bufs=1, space="PSUM"))
            psO = pctx.enter_context(
                tc.tile_pool(name="psO", bufs=1, space="PSUM"))

            def oproj_blocks(tb, obs):
                ts = slice(tb * TB, (tb + 1) * TB)
                final = tb == NTB - 1
                for ob in obs:
                    # last t-block: alternate into the psS ring (dead after
                    # the final exp) to double-buffer the exposed tail
                    pool_ = psS if (final and ob % 2) else psO
                    tag_ = "st" if (final and ob % 2) else "ps_o"
                    ps_o = pool_.tile([P, TB], F32, tag=tag_, name="ps_o")
                    for fc in range(NH):
                        nc.tensor.matmul(
                            ps_o[:], wo_dq[:, fc, ob * P:(ob + 1) * P],
                            y_sb[:, fc, ts],
                            start=(fc == 0), stop=(fc == NH - 1))
                    ot = opool.tile([P, TB], BF, tag="ot", name="ot")
                    if ob % 2 == 0:
                        nc.scalar.copy(ot[:], ps_o[:])
                    else:
                        nc.vector.tensor_copy(ot[:], ps_o[:])
                    nc.sync.dma_start(outT_v[ob, :, ts], ot[:])

            for j in range(NTB):
                ts = slice(j * TB, (j + 1) * TB)
                nkk = 4 * j + 4
                for h in range(NH):
                    qj = qj_all[:, j, h]
                    ps_y = psY.tile([P, TB], F32, tag="py", name="py")
                    exs = sspool.tile([P, TB], BF, tag="exs", name="exs")
                    pend = []  # (kk, ex, off, ncols)

                    def flush_pv(kk, ex, off, ncols):
                        nc.tensor.matmul(ps_y[:, off:],
                                         vhs[:, kk, h * HD:(h + 1) * HD],
                                         ex[:, 0:ncols],
                                         start=(kk == 0), stop=(kk == nkk - 1))

                    for kk in range(nkk):
                        d = kk - 4 * j
                        off = P * d if d >= 0 else 0
                        ncols = TB - off
                        ps_st = psS.tile([P, TB], F32, tag="st", name="st")
                        st = ps_st[:, 0:ncols]
                        nc.tensor.matmul(
                            st, khs[:, h, kk * P:(kk + 1) * P],
                            qj[:, off:TB], start=True, stop=True)
                        ex = expool.tile([P, TB], BF, tag="ex", name="ex")
                        nc.scalar.activation(ex[:, 0:ncols], st, AF.Exp,
                                             scale=SCALE)
                        if d >= 0:
                            # causal mask as post-exp 0/1 multiply: cheaper
                            # (bf16 SBUF in-place) than the -1e30 PSUM add
                            nc.vector.tensor_tensor(ex[:, 0:P], ex[:, 0:P],
                                                    tri[:], ALU.mult)
                        if kk == 0:
                            nc.vector.tensor_copy(exs[:], ex[:])
                        else:
                            nc.vector.tensor_tensor(
                                exs[:, off:], exs[:, off:], ex[:, 0:ncols],
                                ALU.add)
                        pend.append((kk, ex, off, ncols))
                        if len(pend) > 3:
                            flush_pv(*pend.pop(0))
                    while pend:
                        flush_pv(*pend.pop(0))

                    ps_den = psD.tile([1, TB], F32, tag="pd", name="pd")
                    nc.tensor.matmul(ps_den[:], ones_b[:], exs[:],
                                     start=True, stop=True)
                    rec = expool.tile([1, TB], BF, tag="rec", name="rec")
                    with nc.allow_low_precision("bf16 1/denom"):
                        nc.vector.reciprocal(rec[:], ps_den[:])
                    den_b = sspool.tile([P, TB], BF, tag="den_b",
                                        name="den_b")
                    nc.gpsimd.partition_broadcast(den_b[:], rec[:])
                    nc.vector.tensor_tensor(y_sb[:, h, ts], ps_y[:],
                                            den_b[:], ALU.mult)
                oproj_blocks(j, range(G))

    from concourse import library_overlay
    library_overlay.lower_extended_insts(nc)
    return nc


def _rope_tables():
    half = HD // 2
    inv_freq = 1.0 / (10000.0 ** (np.arange(half, dtype=np.float64) / half))
    freqs = np.outer(np.arange(T, dtype=np.float64), inv_freq)  # [T, 64]
    c = np.cos(freqs).T  # [64, T]
    s = np.sin(freqs).T
    # S'' = [sin; -sin]: rope = q*C + swap_halves(q*S'')
    return (np.concatenate([c, c], axis=0),
            np.concatenate([s, -s], axis=0))


def kernel(x, w_q, w_k, w_v, w_o):
    _install_waitfix()
    from concourse.bass_utils import run_bass_kernel_spmd
    import ml_dtypes

    bf16 = ml_dtypes.bfloat16
    x = np.asarray(x, dtype=np.float32)
    w_q = np.asarray(w_q, dtype=np.float32)
    w_k = np.asarray(w_k, dtype=np.float32)
    w_v = np.asarray(w_v, dtype=np.float32)
    w_o = np.asarray(w_o, dtype=np.float32)
    B = x.shape[0]

    if "nc" not in _cached:
        _cached["nc"] = _build_nc()
    nc = _cached["nc"]

    ropeC, ropeS = _rope_tables()
    idx = np.arange(P)
    tri = np.where(idx[:, None] > idx[None, :], np.float32(0.0),
                   np.float32(1.0))

    in_maps = []
    for c in range(8):
        b, hg = divmod(c, 4)
        osl = slice(hg * OSH, (hg + 1) * OSH)
        in_maps.append({
            "xT": np.ascontiguousarray(x[b].T).astype(bf16),
            "wqT": np.ascontiguousarray(w_q[osl, :].T),
            "wkT": np.ascontiguousarray(w_k[osl, :].T),
            "wvT": np.ascontiguousarray(w_v[osl, :].T),
            "woT": np.ascontiguousarray(w_o[:, osl].T),
            "ropeC": ropeC.astype(bf16), "ropeS": ropeS.astype(bf16),
            "tri": tri.astype(bf16),
        })

    import os as _os
    trace = _os.environ.get("BITATTN_TRACE") == "1"
    res = run_bass_kernel_spmd(nc, in_maps, core_ids=list(range(8)),
                               trace=trace)
    _cached["last_res"] = res
    out = np.zeros((B, T, D), dtype=np.float32)
    for c in range(8):
        b = c // 4
        out[b] += res.results[c]["outT"].astype(np.float32).T
    return out
